# revision 1
# baseline (speedup 1.0000x reference)
"""Causal multi-head attention (B=4, T=2048, D=1024, H=16) on 8 NeuronCores.

Sharding:
  stage 1 (QKV proj + attention): core c -> batch c//2, head-group c%2
    (8 of 16 heads, 512 of 1024 channels). Data-parallel on B, tensor-
    parallel on heads.
  stage 2 (output projection): one 8-rank AllToAll re-shards attention
    output to (all 4 batches x 256-token t-slice) per core, then each core
    computes out = attn_out @ W_O.T for its 1024 rows. No reduction needed.

All heavy matmuls run in fp32r (full PE rate, ~19-bit mantissa). exp runs on
the scalar engine reading PSUM directly with the softmax scale fused; the
softmax denominator comes for free as a 65th output row of the PV matmul
(V augmented with a ones column). Causal masking multiplies diagonal-block
probabilities by precomputed 0/1 masks.

The t-chunk loop interleaves projections with attention: after projecting
chunk tc, all k-tiles needed by q-chunk tc exist, so attention for q-chunk tc
runs while the next chunk's projections stream — keeping PE busy during the
ACT-heavy attention phase.
"""
import numpy as np

import concourse.bass as bass
import concourse.mybir as mybir
import concourse.tile as tile
from concourse.bass_utils import run_bass_kernel_spmd

F32 = mybir.dt.float32
F32R = mybir.dt.float32r

P = 128
B, T, D = 4, 2048, 1024
H, HD = 16, 64
NCORES = 8
CH = D // 2          # channels per core (8 heads)
NHP = 4              # head pairs per core
NKT = T // P         # 16 k-tiles
NQC = T // 512       # 4 q-chunks
NIT = D // P         # 8 input-dim tiles
TS256 = 256          # t-slice per core per batch in stage 2


def _split_multiwaits(nc) -> int:
    """walrus here rejects >1 sem wait per instruction; split extras into
    wait-only NoOps on the same engine."""
    nsplit = 0
    for f in nc.m.functions:
        for bb in f.blocks:
            if not any(
                i.sync_info is not None and i.sync_info.on_wait is not None
                and len(i.sync_info.on_wait) > 1 for i in bb.instructions
            ):
                continue
            new_list = []
            for inst in bb.instructions:
                si = inst.sync_info
                if si is not None and si.on_wait is not None and len(si.on_wait) > 1:
                    waits = list(si.on_wait)
                    for k, w in enumerate(waits[:-1]):
                        n = mybir.InstNoOp(
                            name=f"{inst.name}-wsplit{k}", ins=[], outs=[])
                        n.engine = inst.engine
                        n.sync_info = mybir.SyncInfo(on_wait=[w], on_update=[])
                        new_list.append(n)
                        nsplit += 1
                    inst.sync_info = mybir.SyncInfo(
                        on_wait=[waits[-1]], on_update=list(si.on_update or []))
                new_list.append(inst)
            bb.instructions = new_list
    return nsplit


def _build_nc(sim: bool = False, mask_mode: str = "dve"):
    nc = bass.Bass("TRN2", target_bir_lowering=False, debug=False,
                   num_devices=NCORES)
    xt_d = nc.dram_tensor("xt", [D, T], F32R, kind="ExternalInput").ap()
    wq_d = nc.dram_tensor("wq", [D, CH], F32R, kind="ExternalInput").ap()
    wk_d = nc.dram_tensor("wk", [D, CH], F32R, kind="ExternalInput").ap()
    wv_d = nc.dram_tensor("wv", [D, CH], F32R, kind="ExternalInput").ap()
    wo_d = nc.dram_tensor("wo", [D, D], F32R, kind="ExternalInput").ap()
    ones_d = nc.dram_tensor("ones", [P, NKT * NHP * 2], F32R,
                            kind="ExternalInput").ap()
    out_d = nc.dram_tensor("out", [B, 2, P, D], F32, kind="ExternalOutput").ap()
    a2a_in0 = nc.dram_tensor("a2a_in0", [NCORES, CH, P], F32R).ap()
    a2a_out0 = nc.dram_tensor("a2a_out0", [NCORES, CH, P], F32R).ap()
    a2a_in1 = nc.dram_tensor("a2a_in1", [NCORES, CH, P], F32R).ap()
    a2a_out1 = nc.dram_tensor("a2a_out1", [NCORES, CH, P], F32R).ap()

    scale = float(1.0 / np.sqrt(HD))

    with tile.TileContext(nc) as tc:
        with (
            tc.tile_pool(name="persist", bufs=1) as persist,
        ):
            # ---- persistent SBUF tensors -------------------------------
            kt_s = persist.tile([P, NHP, T], F32R)    # K^T  (channels, k)
            va = persist.tile([P, NKT, NHP, 2, HD + 1], F32R)  # V | ones

            with (
                tc.tile_pool(name="wpool", bufs=1) as wpool,
                tc.tile_pool(name="xpool", bufs=1) as xpool,
                tc.tile_pool(name="ob_pool", bufs=2) as ob_pool,
                tc.tile_pool(name="qpool", bufs=2) as qpool,
                tc.tile_pool(name="ao_pool", bufs=2) as ao_pool,
                tc.tile_pool(name="mpool", bufs=1) as mpool,
                tc.tile_pool(name="pt_pool", bufs=4) as pt_pool,
                tc.tile_pool(name="nrm_pool", bufs=1) as nrm_pool,
                tc.tile_pool(name="ppool", bufs=2, space="PSUM") as ppool,
                tc.tile_pool(name="ps_s", bufs=2, space="PSUM") as ps_s,
                tc.tile_pool(name="ps_pv", bufs=1, space="PSUM") as ps_pv,
            ):
                wq = wpool.tile([P, NIT, CH], F32R)
                wk = wpool.tile([P, NIT, CH], F32R)
                wv = wpool.tile([P, NIT, CH], F32R)
                xt_r = xt_d.rearrange("(i p) t -> p i t", p=P)
                xtc0 = xpool.tile([P, NIT, 512], F32R, tag="xtc")
                for it in range(NIT):
                    nc.sync.dma_start(xtc0[:, it], xt_r[:, it, 0:512])
                    nc.sync.dma_start(wv[:, it], wv_d.rearrange(
                        "(i p) o -> p i o", p=P)[:, it])
                for it in range(NIT):
                    nc.sync.dma_start(wq[:, it], wq_d.rearrange(
                        "(i p) o -> p i o", p=P)[:, it])
                    nc.sync.dma_start(wk[:, it], wk_d.rearrange(
                        "(i p) o -> p i o", p=P)[:, it])

                ones64 = mpool.tile([P, 64], F32R)
                nc.sync.dma_start(ones64[:], ones_d[:, 0:64])
                # fill the V|ones denominator column via one broadcast copy
                # (a strided DMA here would be thousands of 4B descriptors)
                nc.scalar.copy(
                    va[:, :, :, :, HD],
                    ones64[:, 0:1].to_broadcast((P, NKT, NHP, 2)))
                masks = []
                if mask_mode == "dve":
                    for i in range(4):
                        m = mpool.tile([P, 512], mybir.dt.bfloat16,
                                       tag=f"mask{i}")
                        nc.gpsimd.memset(m[:], 1.0)
                        nc.gpsimd.affine_select(
                            out=m[:], in_=m[:],
                            compare_op=mybir.AluOpType.is_ge,
                            fill=0.0, base=-P * i, channel_multiplier=-1,
                            pattern=[[1, 512]])
                        masks.append(m)

                # pending projection psum-groups of the NEXT chunk, emitted
                # as PE filler work inside the attention kt loops
                pending = []
                normtail = []

                filler_acc = [0.0]

                def emit_fillers(remaining_units):
                    # proportional pacing: spread the queue across the whole
                    # remaining stage instead of draining it in the first
                    # len(pending) units (late ACT-bound units idle PE)
                    if not pending:
                        return
                    filler_acc[0] += len(pending) / max(1, remaining_units)
                    while filler_acc[0] >= 1.0 and pending:
                        filler_acc[0] -= 1.0
                        pending.pop(0)()

                def project(tc4, xtc=None):
                    """Queue QKV projection psum-groups for t-chunk tc4.
                    Returns the Q^T chunk tile; the groups themselves are
                    emitted later as PE filler inside attention."""
                    if xtc is None:
                        xtc = xpool.tile([P, NIT, 512], F32R, tag="xtc")
                        for it in range(NIT):
                            nc.sync.dma_start(
                                xtc[:, it],
                                xt_r[:, it, tc4 * 512:(tc4 + 1) * 512])
                    qtc = qpool.tile([P, NHP, 512], F32R, tag="qtc")

                    def qk_group(w, dst, dsl, ot):
                        def g():
                            ps = ppool.tile([P, 512], F32, tag="proj")
                            for it in range(NIT):
                                nc.tensor.matmul(
                                    ps[:], w[:, it, ot * P:(ot + 1) * P],
                                    xtc[:, it], start=(it == 0),
                                    stop=(it == NIT - 1))
                            nc.vector.tensor_copy(dst[:, ot, dsl], ps[:])
                        return g

                    def v_group(tt4):
                        def g():
                            ps = ppool.tile([P, 512], F32, tag="proj")
                            for it in range(NIT):
                                nc.tensor.matmul(
                                    ps[:], xtc[:, it, tt4 * P:(tt4 + 1) * P],
                                    wv[:, it], start=(it == 0),
                                    stop=(it == NIT - 1))
                            nc.vector.tensor_copy(
                                va[:, tc4 * 4 + tt4, :, :, 0:HD],
                                ps[:].rearrange("p (hp h d) -> p hp h d",
                                                hp=NHP, h=2))
                        return g

                    if tc4 == 0:
                        # V first: wv+x arrive first and the four V groups
                        # run it-major across four concurrent psums, so each
                        # arriving (x, wv) DMA chunk feeds 4 matmuls instead
                        # of 1 during the DMA-bound startup ramp
                        def v_block0():
                            pss = [
                                ppool.tile([P, 512], F32, tag="proj",
                                           name="v0ps0"),
                                ppool.tile([P, 512], F32, tag="proj",
                                           name="v0ps1"),
                                ps_s.tile([P, 512], F32, tag="s2",
                                          name="v0ps2"),
                                ps_s.tile([P, 512], F32, tag="s2",
                                          name="v0ps3"),
                            ]
                            for it in range(NIT):
                                for tt4 in range(4):
                                    nc.tensor.matmul(
                                        pss[tt4][:],
                                        xtc[:, it, tt4 * P:(tt4 + 1) * P],
                                        wv[:, it], start=(it == 0),
                                        stop=(it == NIT - 1))
                            for tt4 in range(4):
                                nc.vector.tensor_copy(
                                    va[:, tt4, :, :, 0:HD],
                                    pss[tt4][:].rearrange(
                                        "p (hp h d) -> p hp h d",
                                        hp=NHP, h=2))
                        pending.append(v_block0)
                        for ot in range(NHP):
                            pending.append(qk_group(wq, qtc, slice(0, 512), ot))
                            pending.append(qk_group(
                                wk, kt_s,
                                slice(tc4 * 512, (tc4 + 1) * 512), ot))
                    else:
                        for ot in range(NHP):
                            pending.append(qk_group(wq, qtc, slice(0, 512), ot))
                        for ot in range(NHP):
                            pending.append(qk_group(
                                wk, kt_s, slice(tc4 * 512, (tc4 + 1) * 512), ot))
                        for tt4 in range(4):
                            pending.append(v_group(tt4))
                    return qtc

                def attend(hp, qc, qtc, aoq):
                    """Attention for head-pair hp, q-chunk qc. kt loop is
                    software-pipelined: QK(kt+1) issues before PV(kt) so PE
                    isn't stalled behind the exp of the current tile."""
                    nkt = 4 * (qc + 1)
                    pva = ps_pv.tile([HD + 1, 512], F32, tag="pva")
                    pvb = ps_pv.tile([HD + 1, 512], F32, tag="pvb")
                    s2s = {}
                    pts = {}

                    def qk(kt):
                        ksl = slice(kt * P, (kt + 1) * P)
                        f0 = max(0, kt - 4 * qc) * P  # first visible q column
                        s2 = ps_s.tile([P, 1024], F32, tag="s2")
                        nc.tensor.matmul(s2[:, f0:512], kt_s[0:64, hp, ksl],
                                         qtc[0:64, hp, f0:],
                                         start=True, stop=True)
                        nc.tensor.matmul(s2[:, 512 + f0:1024],
                                         kt_s[64:128, hp, ksl],
                                         qtc[64:128, hp, f0:],
                                         start=True, stop=True)
                        s2s[kt] = s2

                    def softmax_pv(kt, remaining):
                        s2 = s2s.pop(kt)
                        pt = pt_pool.tile([P, 2, 512], F32R, tag="pt")
                        di = kt - 4 * qc
                        if mask_mode == "gp":
                            f0 = max(0, di) * P
                            s2v = s2[:].rearrange("p (a b) -> p a b", a=2)
                            nc.scalar.activation(
                                pt[:, :, f0:], s2v[:, :, f0:],
                                mybir.ActivationFunctionType.Exp, scale=scale)
                            if di >= 0:
                                # causal: keep q >= k, zero the rest (incl the
                                # [0:f0) region the restricted exp skipped)
                                nc.gpsimd.affine_select(
                                    out=pt[:], in_=pt[:],
                                    compare_op=mybir.AluOpType.is_ge,
                                    fill=0.0, base=-P * di,
                                    channel_multiplier=-1,
                                    pattern=[[0, 2], [1, 512]])
                        else:
                            # diagonal blocks: only columns >= f0 are causally
                            # visible; exp, mask, and PV all restrict to them
                            # (kt==0 is always full-width, initializing every
                            # PSUM column of the PV accumulators)
                            f0 = max(0, di) * P
                            if f0 > 0:
                                s2v = s2[:].rearrange("p (a b) -> p a b", a=2)
                                nc.scalar.activation(
                                    pt[:, :, f0:], s2v[:, :, f0:],
                                    mybir.ActivationFunctionType.Exp,
                                    scale=scale)
                            else:
                                nc.scalar.activation(
                                    pt[:].rearrange("p a b -> p (a b)"), s2[:],
                                    mybir.ActivationFunctionType.Exp,
                                    scale=scale)
                            if di >= 0:
                                nc.vector.tensor_mul(
                                    pt[:, :, f0:], pt[:, :, f0:],
                                    masks[di][:, None, f0:].to_broadcast(
                                        (P, 2, 512 - f0)))
                        f0 = max(0, di) * P
                        nc.tensor.matmul(pva[:, f0:], va[:, kt, hp, 0],
                                         pt[:, 0, f0:],
                                         start=(kt == 0), stop=(kt == nkt - 1))
                        nc.tensor.matmul(pvb[:, f0:], va[:, kt, hp, 1],
                                         pt[:, 1, f0:],
                                         start=(kt == 0), stop=(kt == nkt - 1))
                        if kt >= 2 and normtail:
                            normtail.pop(0)()
                        emit_fillers(remaining)

                    qk(0)
                    for kt in range(1, nkt):
                        qk(kt)
                        softmax_pv(kt - 1, (nkt - kt) + (NHP - 1 - hp) * nkt)
                    softmax_pv(nkt - 1, 1 + (NHP - 1 - hp) * nkt)

                    # copy PV accumulators out of PSUM fast (frees banks);
                    # defer the recip->broadcast->scale tail into the next
                    # head-pair's kt loop so PE never stalls behind it
                    pvs = nrm_pool.tile([P, 2, 512], F32, tag="pvs")
                    nc.vector.tensor_copy(pvs[0:65, 0], pva[:])
                    if hp == NHP - 1 and qc in (1, 3):
                        # stage-final tail gates a collective launch and runs
                        # with ACT idle: split the copies across engines to
                        # shorten the serial chain
                        nc.scalar.copy(pvs[0:65, 1], pvb[:])
                    else:
                        nc.vector.tensor_copy(pvs[0:65, 1], pvb[:])
                    rden = nrm_pool.tile([P, 2, 512], F32R, tag="rden")
                    with nc.allow_low_precision("f32r softmax denominators"):
                        nc.vector.reciprocal(rden[64:65, 0], pvs[64:65, 0])
                        nc.vector.reciprocal(rden[64:65, 1], pvs[64:65, 1])

                    def tail(hp=hp, pvs=pvs, rden=rden):
                        rba = ppool.tile([64, 512], F32, tag="proj")
                        rbb = ppool.tile([64, 512], F32, tag="proj")
                        nc.tensor.matmul(rba[:], ones64[64:65, :],
                                         rden[64:65, 0], start=True, stop=True)
                        nc.tensor.matmul(rbb[:], ones64[64:65, :],
                                         rden[64:65, 1], start=True, stop=True)
                        nc.vector.tensor_mul(aoq[0:64, hp], pvs[0:64, 0],
                                             rba[:])
                        nc.vector.tensor_mul(aoq[64:128, hp], pvs[0:64, 1],
                                             rbb[:])
                        # ship this head-pair's slice to the exchange buffer
                        # immediately so the collective's inputs aren't gated
                        # on one bulk DMA burst at stage end
                        nc.sync.dma_start(
                            a2a_r[qc // 2][:, hp, (qc % 2) * 4:(qc % 2) * 4 + 4],
                            aoq[:, hp].rearrange("p (j t) -> p j t", j=4))
                    normtail.append(tail)

                # interleaved: project chunk tc, then attention q-chunk tc,
                # streaming each finished chunk into the re-shard buffers.
                # stage-2 row owner of q = m*1024 + j*128 + p is core j, so
                # the first collective can fire once q < 1024 is done.
                a2a_r = [a.rearrange("j (hp p) t -> p hp j t", p=P)
                         for a in (a2a_in0, a2a_in1)]

                def emit_collective(m):
                    cin = (a2a_in0, a2a_in1)[m]
                    cout = (a2a_out0, a2a_out1)[m]
                    if sim:
                        nc.sync.dma_start(cout, cin)
                    else:
                        nc.gpsimd.collective_compute(
                            "AllToAll", mybir.AluOpType.bypass,
                            replica_groups=[list(range(NCORES))],
                            ins=[cin], outs=[cout])

                qtc = project(0, xtc=xtc0)
                while pending:
                    pending.pop(0)()
                for tc4 in range(NQC):
                    if tc4 + 1 < NQC:
                        next_qtc = project(tc4 + 1)  # queued as fillers
                    aoq = ao_pool.tile([P, NHP, 512], F32R, tag="aoq")
                    for hp in range(NHP):
                        attend(hp, tc4, qtc, aoq)
                    if tc4 in (1, 3):
                        # drains are only load-bearing before a collective
                        # launch; elsewhere tails/fillers spill into the next
                        # stage's kt loops for smoother boundaries
                        while normtail:
                            normtail.pop(0)()
                        while pending:
                            pending.pop(0)()
                    if tc4 == 1:
                        emit_collective(0)
                    if tc4 == 2:
                        # chunk-3 projections are queued, so the wq/wk pool
                        # slots retire after them; reuse them for W_O and
                        # queue the m=0 output projection as qc3 filler work
                        # (its AllToAll finished during qc2's attention)
                        wo0 = wpool.tile([P, NIT, 512], F32R, tag="wq")
                        wo1 = wpool.tile([P, NIT, 512], F32R, tag="wk")
                        wo_r = wo_d.rearrange("(i p) o -> p i o", p=P)

                        def wo_dma(w, oc):
                            def g():
                                for it in range(NIT):
                                    nc.sync.dma_start(
                                        w[:, it],
                                        wo_r[:, it, oc * 512:(oc + 1) * 512])
                            return g

                        def o_group(b, m, aob, osb, w, oc, cout_idx):
                            def g():
                                ps = ppool.tile([P, 512], F32, tag="proj")
                                for ct in range(NIT):
                                    nc.tensor.matmul(
                                        ps[:], aob[:, ct], w[:, ct],
                                        start=(ct == 0), stop=(ct == NIT - 1))
                                nc.vector.tensor_copy(
                                    osb[:, oc * 512:(oc + 1) * 512], ps[:])
                                if oc == 1:
                                    nc.sync.dma_start(out_d[b, m], osb[:])
                            return g

                        def o_stage(b, m, cout):
                            def g():
                                aob = ob_pool.tile([P, NIT, P], F32R,
                                                   tag="aob")
                                osb = ob_pool.tile([P, D], F32, tag="osb")
                                nc.sync.dma_start(
                                    aob[:],
                                    cout[2 * b:2 * b + 2].rearrange(
                                        "s (c p) t -> p (s c) t", p=P))
                                pending.append(
                                    o_group(b, m, aob, osb, wo0, 0, None))
                                pending.append(
                                    o_group(b, m, aob, osb, wo1, 1, None))
                            return g

                        pending.append(wo_dma(wo0, 0))
                        pending.append(wo_dma(wo1, 1))
                        for b in range(B):
                            pending.append(o_stage(b, 0, a2a_out0))
                    if tc4 + 1 < NQC:
                        qtc = next_qtc
                emit_collective(1)

                # ---- m=1 output projection (tail) ----------------------
                for b in range(B):
                    aob = ob_pool.tile([P, NIT, P], F32R, tag="aob")
                    osb = ob_pool.tile([P, D], F32, tag="osb")
                    nc.sync.dma_start(
                        aob[:],
                        a2a_out1[2 * b:2 * b + 2].rearrange(
                            "s (c p) t -> p (s c) t", p=P))
                    for oc in range(2):
                        w = (wo0, wo1)[oc]
                        ps = ppool.tile([P, 512], F32, tag="proj")
                        for ct in range(NIT):
                            nc.tensor.matmul(
                                ps[:], aob[:, ct], w[:, ct],
                                start=(ct == 0), stop=(ct == NIT - 1))
                        nc.vector.tensor_copy(
                            osb[:, oc * 512:(oc + 1) * 512], ps[:])
                    nc.sync.dma_start(out_d[b, 1], osb[:])

    _split_multiwaits(nc)
    return nc


_NC_CACHE = None


def _get_nc():
    global _NC_CACHE
    if _NC_CACHE is None:
        _NC_CACHE = _build_nc()
    return _NC_CACHE


def make_in_maps(x, W_Q, W_K, W_V, W_O):
    wqt = np.ascontiguousarray(W_Q.T)
    wkt = np.ascontiguousarray(W_K.T)
    wvt = np.ascontiguousarray(W_V.T)
    wot = np.ascontiguousarray(W_O.T)
    ones = np.ones((P, NKT * NHP * 2), np.float32)
    in_maps = []
    for c in range(NCORES):
        b, g = c // 2, c % 2
        in_maps.append({
            "xt": np.ascontiguousarray(x[b].T),
            "wq": np.ascontiguousarray(wqt[:, g * CH:(g + 1) * CH]),
            "wk": np.ascontiguousarray(wkt[:, g * CH:(g + 1) * CH]),
            "wv": np.ascontiguousarray(wvt[:, g * CH:(g + 1) * CH]),
            "wo": wot,
            "ones": ones,
        })
    return in_maps


def assemble(results):
    out = np.empty((B, T, D), np.float32)
    for j in range(NCORES):
        o = results[j]["out"]  # [B, 2, 128, D]
        for b in range(B):
            for m in range(2):
                r0 = m * 1024 + j * P
                out[b, r0:r0 + P, :] = o[b, m]
    return out


def kernel(x, W_Q, W_K, W_V, W_O):
    x = np.asarray(x, np.float32)
    in_maps = make_in_maps(x, np.asarray(W_Q, np.float32),
                           np.asarray(W_K, np.float32),
                           np.asarray(W_V, np.float32),
                           np.asarray(W_O, np.float32))
    nc = _get_nc()
    res = run_bass_kernel_spmd(nc, in_maps, core_ids=list(range(NCORES)))
    return assemble(res.results)



# revision 24
# speedup vs baseline: 1.1231x; 1.1231x over previous
"""Causal multi-head attention (B=4, T=2048, D=1024, H=16) on 8 NeuronCores.

Sharding:
  stage 1 (QKV proj + attention): core c -> batch c//2, head-group c%2
    (8 of 16 heads, 512 of 1024 channels). Data-parallel on B, tensor-
    parallel on heads.
  stage 2 (output projection): one 8-rank AllToAll re-shards attention
    output to (all 4 batches x 256-token t-slice) per core, then each core
    computes out = attn_out @ W_O.T for its 1024 rows. No reduction needed.

Matmul operands are bf16 (same PE rate as fp32r at >=256 moving columns,
full rate below it, half the DMA/SBUF footprint); PSUM accumulation stays
fp32. exp runs on the scalar engine reading PSUM directly with the softmax
scale fused; the softmax denominator comes for free as a 65th output row of
the PV matmul (V augmented with a ones column). Causal masking multiplies
diagonal-block probabilities by precomputed 0/1 masks. The reciprocal
denominators are partition-broadcast by a small SBUF->SBUF DMA instead of a
PE ones-matmul.

Scheduling: attention is ACT-bound per k-tile (exp ~1040ns vs QK+PV
~850ns), so projection work for the NEXT chunk is queued and paced into the
attention stream as PE filler. The filler queue is labeled with
(chunk, kind, index) and force-drained at each consumer site (Q before the
attend that reads it, K/V before the k-tile that reads them), which lets the
pacer spread filler across the whole remaining schedule without correctness
risk. The m=0 output projection is deferred until after the second
collective is emitted so its PE work hides the collective+reshard latency
that gates the m=1 output projection.
"""
import numpy as np
import ml_dtypes

import concourse.bass as bass
import concourse.mybir as mybir
import concourse.tile as tile
from concourse.bass_utils import run_bass_kernel_spmd

F32 = mybir.dt.float32
F32R = mybir.dt.float32r
BF16 = mybir.dt.bfloat16

P = 128
B, T, D = 4, 2048, 1024
H, HD = 16, 64
NCORES = 8
CH = D // 2          # channels per core (8 heads)
NHP = 4              # head pairs per core
NKT = T // P         # 16 k-tiles
NQC = T // 512       # 4 q-chunks
NIT = D // P         # 8 input-dim tiles


def _split_multiwaits(nc) -> int:
    """walrus here rejects >1 sem wait per instruction; split extras into
    wait-only NoOps on the same engine."""
    nsplit = 0
    for f in nc.m.functions:
        for bb in f.blocks:
            if not any(
                i.sync_info is not None and i.sync_info.on_wait is not None
                and len(i.sync_info.on_wait) > 1 for i in bb.instructions
            ):
                continue
            new_list = []
            for inst in bb.instructions:
                si = inst.sync_info
                if si is not None and si.on_wait is not None and len(si.on_wait) > 1:
                    waits = list(si.on_wait)
                    for k, w in enumerate(waits[:-1]):
                        n = mybir.InstNoOp(
                            name=f"{inst.name}-wsplit{k}", ins=[], outs=[])
                        n.engine = inst.engine
                        n.sync_info = mybir.SyncInfo(on_wait=[w], on_update=[])
                        new_list.append(n)
                        nsplit += 1
                    inst.sync_info = mybir.SyncInfo(
                        on_wait=[waits[-1]], on_update=list(si.on_update or []))
                new_list.append(inst)
            bb.instructions = new_list
    return nsplit


def _build_nc(sim: bool = False):
    nc = bass.Bass("TRN2", target_bir_lowering=False, debug=False,
                   num_devices=NCORES)
    xt_d = nc.dram_tensor("xt", [D, T], BF16, kind="ExternalInput").ap()
    wq_d = nc.dram_tensor("wq", [D, CH], BF16, kind="ExternalInput").ap()
    wk_d = nc.dram_tensor("wk", [D, CH], BF16, kind="ExternalInput").ap()
    wv_d = nc.dram_tensor("wv", [D, CH], BF16, kind="ExternalInput").ap()
    wo_d = nc.dram_tensor("wo", [D, D], BF16, kind="ExternalInput").ap()
    out_d = nc.dram_tensor("out", [B, 2, P, D], F32, kind="ExternalOutput").ap()
    a2a_in0 = nc.dram_tensor("a2a_in0", [NCORES, CH, P], BF16).ap()
    a2a_out0 = nc.dram_tensor("a2a_out0", [NCORES, CH, P], BF16).ap()
    a2a_in1 = nc.dram_tensor("a2a_in1", [NCORES, CH, P], BF16).ap()
    a2a_out1 = nc.dram_tensor("a2a_out1", [NCORES, CH, P], BF16).ap()

    scale = float(1.0 / np.sqrt(HD))
    # attention units (k-tiles) per q-chunk, for filler pacing
    UNITS = [4 * (qc + 1) * NHP for qc in range(NQC)]

    with tile.TileContext(nc) as tc:
        with (
            tc.tile_pool(name="persist", bufs=1) as persist,
        ):
            # ---- persistent SBUF tensors -------------------------------
            kt_s = persist.tile([P, NHP, T], BF16)    # K^T  (channels, k)
            va = persist.tile([P, NKT, NHP, 2, HD + 1], BF16)  # V | ones

            with (
                tc.tile_pool(name="wpool", bufs=1) as wpool,
                tc.tile_pool(name="xpool", bufs=2) as xpool,
                tc.tile_pool(name="aob_pool", bufs=8) as aob_pool,
                tc.tile_pool(name="osb_pool", bufs=2) as osb_pool,
                tc.tile_pool(name="qpool", bufs=2) as qpool,
                tc.tile_pool(name="ao_pool", bufs=2) as ao_pool,
                tc.tile_pool(name="mpool", bufs=1) as mpool,
                tc.tile_pool(name="pt_pool", bufs=4) as pt_pool,
                tc.tile_pool(name="nrm_pool", bufs=1) as nrm_pool,
                tc.tile_pool(name="ppool", bufs=2, space="PSUM") as ppool,
                tc.tile_pool(name="ps_s", bufs=2, space="PSUM") as ps_s,
                tc.tile_pool(name="ps_pv", bufs=1, space="PSUM") as ps_pv,
            ):
                wq = wpool.tile([P, NIT, CH], BF16)
                wk = wpool.tile([P, NIT, CH], BF16)
                wv = wpool.tile([P, NIT, CH], BF16)
                wo0 = wpool.tile([P, NIT, 512], BF16)
                wo1 = wpool.tile([P, NIT, 512], BF16)
                xt_r = xt_d.rearrange("(i p) t -> p i t", p=P)
                xtc0 = xpool.tile([P, NIT, 512], BF16, tag="xtc")
                # the it=0 slices arrive as small pieces so the very first
                # matmuls are not gated on full 128KB transfers
                wv_r = wv_d.rearrange("(i p) o -> p i o", p=P)
                nc.sync.dma_start(xtc0[:, 0, 0:128], xt_r[:, 0, 0:128])
                nc.sync.dma_start(wv[:, 0, 0:256], wv_r[:, 0, 0:256])
                nc.sync.dma_start(xtc0[:, 0, 128:512], xt_r[:, 0, 128:512])
                nc.sync.dma_start(wv[:, 0, 256:512], wv_r[:, 0, 256:512])
                # batched remainders: one descriptor-heavy DMA beats eight
                # instruction-overhead-bound ones (fixed HWDGE cost per DMA)
                nc.sync.dma_start(xtc0[:, 1:], xt_r[:, 1:, 0:512])
                nc.sync.dma_start(wv[:, 1:], wv_r[:, 1:])
                nc.sync.dma_start(wq[:], wq_d.rearrange(
                    "(i p) o -> p i o", p=P))
                nc.sync.dma_start(wk[:], wk_d.rearrange(
                    "(i p) o -> p i o", p=P))

                # ones: stationary rows for the denominator broadcast
                # matmuls + broadcast-copy source for the V|ones column
                # (a strided bf16 memset into va fails the ISA memset
                # value-type check, so fill via ACT broadcast copy)
                ones64 = mpool.tile([P, 64], BF16, tag="ones64")
                nc.gpsimd.memset(ones64[:], 1.0)
                nc.scalar.copy(
                    va[:, :, :, :, HD],
                    ones64[:, 0:1].to_broadcast((P, NKT, NHP, 2)))
                masks = []
                for i in range(4):
                    m = mpool.tile([P, 512], BF16, tag=f"mask{i}")
                    nc.gpsimd.memset(m[:], 1.0)
                    nc.gpsimd.affine_select(
                        out=m[:], in_=m[:],
                        compare_op=mybir.AluOpType.is_ge,
                        fill=0.0, base=-P * i, channel_multiplier=-1,
                        pattern=[[1, 512]])
                    masks.append(m)

                # pending projection psum-groups of upcoming chunks, emitted
                # as PE filler work inside the attention kt loops. Entries
                # are (label, fn) with label=(chunk, kind, idx) so consumer
                # sites can force-drain exactly what they depend on.
                pending = []
                normtail = []
                filler_acc = [0.0]
                reserve = [0]   # entries kept back for the post-collective gap

                def emit_fillers(remaining_units, boost=0.0):
                    # proportional pacing: spread the queue across the whole
                    # remaining schedule instead of draining it greedily
                    # (late ACT-bound units would idle PE). `boost` forces
                    # extra pops at known PE-stall sites.
                    if len(pending) <= reserve[0]:
                        return
                    filler_acc[0] += boost + len(pending) / max(
                        1, remaining_units)
                    while filler_acc[0] >= 1.0 and len(pending) > reserve[0]:
                        filler_acc[0] -= 1.0
                        pending.pop(0)[1]()

                def force_drain(chunk, kind, idx):
                    """Pop fillers (in order) until no queued entry matches
                    (chunk, kind, <=idx) — consumer is about to read them."""
                    while any(lb[0] == chunk and lb[1] == kind and lb[2] <= idx
                              for lb, _ in pending):
                        pending.pop(0)[1]()

                def project(tc4, xtc=None):
                    """Queue QKV projection psum-groups for t-chunk tc4.
                    Returns the Q^T chunk tile; the groups themselves are
                    emitted later as PE filler inside attention."""
                    if xtc is None:
                        xtc = xpool.tile([P, NIT, 512], BF16, tag="xtc")
                        nc.sync.dma_start(
                            xtc[:], xt_r[:, :, tc4 * 512:(tc4 + 1) * 512])
                    qtc = qpool.tile([P, NHP, 512], BF16, tag="qtc")

                    def qk_group(w, dst, dsl, ot):
                        def g():
                            ps = ppool.tile([P, 512], F32, tag="proj")
                            for it in range(NIT):
                                nc.tensor.matmul(
                                    ps[:], w[:, it, ot * P:(ot + 1) * P],
                                    xtc[:, it], start=(it == 0),
                                    stop=(it == NIT - 1))
                            nc.vector.tensor_copy(dst[:, ot, dsl], ps[:])
                        return g

                    def v_group(tt4):
                        def g():
                            ps = ppool.tile([P, 512], F32, tag="proj")
                            for it in range(NIT):
                                nc.tensor.matmul(
                                    ps[:], xtc[:, it, tt4 * P:(tt4 + 1) * P],
                                    wv[:, it], start=(it == 0),
                                    stop=(it == NIT - 1))
                            nc.vector.tensor_copy(
                                va[:, tc4 * 4 + tt4, :, :, 0:HD],
                                ps[:].rearrange("p (hp h d) -> p hp h d",
                                                hp=NHP, h=2))
                        return g

                    if tc4 == 0:
                        # V first: wv+x arrive first and the four V groups
                        # run it-major across four concurrent psums, so each
                        # arriving (x, wv) DMA chunk feeds 4 matmuls instead
                        # of 1 during the DMA-bound startup ramp
                        def v_block0():
                            pss = [
                                ppool.tile([P, 512], F32, tag="proj",
                                           name="v0ps0"),
                                ppool.tile([P, 512], F32, tag="proj",
                                           name="v0ps1"),
                                ps_s.tile([P, 512], F32, tag="s2",
                                          name="v0ps2"),
                                ps_s.tile([P, 512], F32, tag="s2",
                                          name="v0ps3"),
                            ]
                            for it in range(NIT):
                                for tt4 in range(4):
                                    nc.tensor.matmul(
                                        pss[tt4][:],
                                        xtc[:, it, tt4 * P:(tt4 + 1) * P],
                                        wv[:, it], start=(it == 0),
                                        stop=(it == NIT - 1))
                            for tt4 in range(4):
                                nc.vector.tensor_copy(
                                    va[:, tt4, :, :, 0:HD],
                                    pss[tt4][:].rearrange(
                                        "p (hp h d) -> p hp h d",
                                        hp=NHP, h=2))
                        pending.append(((0, "v", 3), v_block0))
                        for ot in range(NHP):
                            pending.append(((0, "q", ot), qk_group(
                                wq, qtc, slice(0, 512), ot)))
                            pending.append(((0, "k", ot), qk_group(
                                wk, kt_s,
                                slice(tc4 * 512, (tc4 + 1) * 512), ot)))
                    else:
                        # interleave Q/K per head-pair so force-drain
                        # deadlines pop the minimum prefix
                        for ot in range(NHP):
                            pending.append(((tc4, "q", ot), qk_group(
                                wq, qtc, slice(0, 512), ot)))
                            pending.append(((tc4, "k", ot), qk_group(
                                wk, kt_s,
                                slice(tc4 * 512, (tc4 + 1) * 512), ot)))
                        for tt4 in range(4):
                            pending.append(((tc4, "v", tt4), v_group(tt4)))
                    return qtc

                def attend(hp, qc, qtc, aoq, rem_after):
                    """Attention for head-pair hp, q-chunk qc. kt loop is
                    software-pipelined: QK(kt+1) issues before PV(kt) so PE
                    isn't stalled behind the exp of the current tile."""
                    nkt = 4 * (qc + 1)
                    force_drain(qc, "q", hp)
                    pva = ps_pv.tile([HD + 1, 512], F32, tag="pva")
                    pvb = ps_pv.tile([HD + 1, 512], F32, tag="pvb")
                    s2s = {}

                    def qk(kt):
                        force_drain(kt // 4, "k", hp)
                        ksl = slice(kt * P, (kt + 1) * P)
                        f0 = max(0, kt - 4 * qc) * P  # first visible q column
                        s2 = ps_s.tile([P, 1024], F32, tag="s2")
                        nc.tensor.matmul(s2[:, f0:512], kt_s[0:64, hp, ksl],
                                         qtc[0:64, hp, f0:],
                                         start=True, stop=True)
                        nc.tensor.matmul(s2[:, 512 + f0:1024],
                                         kt_s[64:128, hp, ksl],
                                         qtc[64:128, hp, f0:],
                                         start=True, stop=True)
                        s2s[kt] = s2

                    def softmax_pv(kt, remaining):
                        force_drain(kt // 4, "v", kt % 4)
                        s2 = s2s.pop(kt)
                        pt = pt_pool.tile([P, 2, 512], BF16, tag="pt")
                        di = kt - 4 * qc
                        # diagonal blocks: only columns >= f0 are causally
                        # visible; exp, mask, and PV all restrict to them
                        # (kt==0 is always full-width, initializing every
                        # PSUM column of the PV accumulators)
                        f0 = max(0, di) * P
                        if f0 > 0:
                            s2v = s2[:].rearrange("p (a b) -> p a b", a=2)
                            nc.scalar.activation(
                                pt[:, :, f0:], s2v[:, :, f0:],
                                mybir.ActivationFunctionType.Exp,
                                scale=scale)
                        else:
                            nc.scalar.activation(
                                pt[:].rearrange("p a b -> p (a b)"), s2[:],
                                mybir.ActivationFunctionType.Exp,
                                scale=scale)
                        if di >= 0:
                            # only the 128-col diagonal block needs masking;
                            # columns beyond it are fully causally visible
                            f1 = f0 + P
                            nc.vector.tensor_mul(
                                pt[:, :, f0:f1], pt[:, :, f0:f1],
                                masks[di][:, None, f0:f1].to_broadcast(
                                    (P, 2, P)))
                        nc.tensor.matmul(pva[:, f0:], va[:, kt, hp, 0],
                                         pt[:, 0, f0:],
                                         start=(kt == 0), stop=(kt == nkt - 1))
                        nc.tensor.matmul(pvb[:, f0:], va[:, kt, hp, 1],
                                         pt[:, 1, f0:],
                                         start=(kt == 0), stop=(kt == nkt - 1))
                        if kt >= 2 and normtail:
                            normtail.pop(0)()
                        emit_fillers(remaining)

                    qk(0)
                    for kt in range(1, nkt):
                        qk(kt)
                        if kt == 1:
                            # cross-attend boundary: PV(0) waits on exp(0)
                            # and qk(2) on the s2 slot it frees — nothing
                            # attention-side can run, so force one filler in
                            emit_fillers(nkt - 1 + rem_after, boost=0.9)
                        softmax_pv(kt - 1, (nkt - kt) + rem_after)
                    softmax_pv(nkt - 1, 1 + rem_after)

                    final = hp == NHP - 1 and qc in (1, 3)
                    pvs = nrm_pool.tile([P, 2, 512], F32, tag="pvs")
                    rden = nrm_pool.tile([P, 2, 512], BF16, tag="rden")
                    if final:
                        # this tail gates a collective launch: shortest
                        # possible chain — reciprocals read the PSUM
                        # denominator rows directly and the copies split
                        # across DVE/ACT
                        with nc.allow_low_precision("f32r softmax denoms"):
                            nc.vector.reciprocal(rden[64:65, 0], pva[64:65])
                            nc.vector.reciprocal(rden[64:65, 1], pvb[64:65])
                        nc.vector.tensor_copy(pvs[0:65, 0], pva[:])
                        nc.scalar.copy(pvs[0:65, 1], pvb[:])
                        rba = ppool.tile([64, 512], F32, tag="proj")
                        rbb = ppool.tile([64, 512], F32, tag="proj")
                        nc.tensor.matmul(rba[:], ones64[64:65, :],
                                         rden[64:65, 0],
                                         start=True, stop=True)
                        nc.tensor.matmul(rbb[:], ones64[64:65, :],
                                         rden[64:65, 1],
                                         start=True, stop=True)
                        nc.vector.tensor_mul(aoq[0:64, hp], pvs[0:64, 0],
                                             rba[:])
                        nc.vector.tensor_mul(aoq[64:128, hp], pvs[0:64, 1],
                                             rbb[:])
                        nc.sync.dma_start(
                            a2a_r[qc // 2][0:64, hp,
                                           (qc % 2) * 4:(qc % 2) * 4 + 4],
                            aoq[0:64, hp].rearrange("p (j t) -> p j t", j=4))
                        nc.sync.dma_start(
                            a2a_r[qc // 2][64:128, hp,
                                           (qc % 2) * 4:(qc % 2) * 4 + 4],
                            aoq[64:128, hp].rearrange("p (j t) -> p j t",
                                                      j=4))
                        return
                    # copy PV accumulators out of PSUM fast (frees banks);
                    # defer the recip->broadcast->scale tail into the next
                    # head-pair's kt loop so PE never stalls behind it
                    nc.vector.tensor_copy(pvs[0:65, 0], pva[:])
                    nc.vector.tensor_copy(pvs[0:65, 1], pvb[:])
                    with nc.allow_low_precision("f32r softmax denominators"):
                        nc.vector.reciprocal(rden[64:65, 0], pvs[64:65, 0])
                        nc.vector.reciprocal(rden[64:65, 1], pvs[64:65, 1])

                    def tail(hp=hp, pvs=pvs, rden=rden):
                        rba = ppool.tile([64, 512], F32, tag="proj")
                        rbb = ppool.tile([64, 512], F32, tag="proj")
                        nc.tensor.matmul(rba[:], ones64[64:65, :],
                                         rden[64:65, 0],
                                         start=True, stop=True)
                        nc.tensor.matmul(rbb[:], ones64[64:65, :],
                                         rden[64:65, 1],
                                         start=True, stop=True)
                        nc.vector.tensor_mul(aoq[0:64, hp], pvs[0:64, 0],
                                             rba[:])
                        nc.vector.tensor_mul(aoq[64:128, hp], pvs[0:64, 1],
                                             rbb[:])
                        # ship this head-pair's slice to the exchange buffer
                        # immediately so the collective's inputs aren't gated
                        # on one bulk DMA burst at stage end
                        nc.sync.dma_start(
                            a2a_r[qc // 2][:, hp, (qc % 2) * 4:(qc % 2) * 4 + 4],
                            aoq[:, hp].rearrange("p (j t) -> p j t", j=4))
                    normtail.append(tail)

                # interleaved: project chunk tc, then attention q-chunk tc,
                # streaming each finished chunk into the re-shard buffers.
                # stage-2 row owner of q = m*1024 + j*128 + p is core j, so
                # the first collective can fire once q < 1024 is done.
                a2a_r = [a.rearrange("j (hp p) t -> p hp j t", p=P)
                         for a in (a2a_in0, a2a_in1)]

                def emit_collective(m):
                    cin = (a2a_in0, a2a_in1)[m]
                    cout = (a2a_out0, a2a_out1)[m]
                    if sim:
                        nc.sync.dma_start(cout, cin)
                    else:
                        nc.gpsimd.collective_compute(
                            "AllToAll", mybir.AluOpType.bypass,
                            replica_groups=[list(range(NCORES))],
                            ins=[cin], outs=[cout])

                def load_aob(b, cout):
                    aob = aob_pool.tile([P, NIT, P], BF16, tag="aob")
                    nc.sync.dma_start(
                        aob[:],
                        cout[2 * b:2 * b + 2].rearrange(
                            "s (c p) t -> p (s c) t", p=P))
                    return aob

                def o_group(b, m, aob, osb, w, oc):
                    ps = ppool.tile([P, 512], F32, tag="proj")
                    for ct in range(NIT):
                        nc.tensor.matmul(
                            ps[:], aob[:, ct], w[:, ct],
                            start=(ct == 0), stop=(ct == NIT - 1))
                    osl = slice(oc * 512, (oc + 1) * 512)
                    nc.vector.tensor_copy(osb[:, osl], ps[:])
                    # m=1 stores per half (shortest final chain); m=0 stores
                    # whole tiles (fewer fixed-cost DMA slots competing with
                    # the collective and reshard loads)
                    if m == 1:
                        nc.sync.dma_start(out_d[b, m, :, osl], osb[:, osl])
                    elif oc == 1:
                        nc.sync.dma_start(out_d[b, m], osb[:])

                def o_stage(b, m, aob_get):
                    """Two labeled filler entries (one per 512-col half of
                    W_O) sharing one staging tile."""
                    slot = {}

                    def g(oc):
                        def f():
                            if "osb" not in slot:
                                slot["osb"] = osb_pool.tile(
                                    [P, D], F32, tag="osb",
                                    name=f"osb{m}_{b}")
                            o_group(b, m, aob_get(b), slot["osb"],
                                    (wo0, wo1)[oc], oc)
                        return f
                    return [((9, "o", 2 * b + oc), g(oc)) for oc in range(2)]

                aob0 = {}
                qtc = project(0, xtc=xtc0)
                # chunk 0 must fully project before attention starts
                while pending:
                    pending.pop(0)[1]()
                for tc4 in range(NQC):
                    if tc4 + 1 < NQC:
                        next_qtc = project(tc4 + 1)  # queued as fillers
                    aoq = ao_pool.tile([P, NHP, 512], BF16, tag="aoq")
                    for hp in range(NHP):
                        rem_after = (NHP - 1 - hp) * 4 * (tc4 + 1) + sum(
                            UNITS[tc4 + 1:])
                        attend(hp, tc4, qtc, aoq, rem_after)
                    if tc4 in (1, 3):
                        # the collective launch is gated on every tail DMA
                        # of its half; drain them now (fillers keep pacing)
                        while normtail:
                            normtail.pop(0)()
                    if tc4 == 1:
                        emit_collective(0)
                    if tc4 == 2:
                        # qc0/qc1 results finished resharding during qc2:
                        # W_O + the m=0 reshard tiles stream in now. The m=0
                        # output projection joins the filler queue behind
                        # chunk-3's projections, but at least 4 groups are
                        # held back (reserve) so PE has work after
                        # collective 1 is emitted, hiding the
                        # collective+reshard latency that gates m=1.
                        wo_r = wo_d.rearrange("(i p) o -> p i o", p=P)
                        nc.sync.dma_start(wo0[:], wo_r[:, :, 0:512])
                        nc.sync.dma_start(wo1[:], wo_r[:, :, 512:1024])
                        for b in range(B):
                            aob0[b] = load_aob(b, a2a_out0)
                        for b in range(B):
                            pending.extend(o_stage(b, 0, aob0.get))
                        reserve[0] = 8
                    if tc4 + 1 < NQC:
                        qtc = next_qtc
                while len(pending) > reserve[0]:
                    pending.pop(0)[1]()
                emit_collective(1)

                # m=1 reshard loads issue first: they only wait on the
                # collective, and queueing them behind the m=0 stores would
                # delay them on the in-order DMA queue
                aob1 = {}
                for b in range(B):
                    aob1[b] = load_aob(b, a2a_out1)

                # reserved m=0 groups hide the collective+reshard latency
                reserve[0] = 0
                while pending:
                    pending.pop(0)[1]()

                # ---- m=1 output projection (tail) ----------------------
                for b in range(B):
                    for lbl, f in o_stage(b, 1, aob1.get):
                        f()

    _split_multiwaits(nc)
    return nc


_NC_CACHE = None


def _get_nc():
    global _NC_CACHE
    if _NC_CACHE is None:
        _NC_CACHE = _build_nc()
    return _NC_CACHE


def make_in_maps(x, W_Q, W_K, W_V, W_O):
    bf = ml_dtypes.bfloat16
    wqt = np.ascontiguousarray(W_Q.T).astype(bf)
    wkt = np.ascontiguousarray(W_K.T).astype(bf)
    wvt = np.ascontiguousarray(W_V.T).astype(bf)
    wot = np.ascontiguousarray(W_O.T).astype(bf)
    in_maps = []
    for c in range(NCORES):
        b, g = c // 2, c % 2
        in_maps.append({
            "xt": np.ascontiguousarray(x[b].T).astype(bf),
            "wq": np.ascontiguousarray(wqt[:, g * CH:(g + 1) * CH]),
            "wk": np.ascontiguousarray(wkt[:, g * CH:(g + 1) * CH]),
            "wv": np.ascontiguousarray(wvt[:, g * CH:(g + 1) * CH]),
            "wo": wot,
        })
    return in_maps


def assemble(results):
    out = np.empty((B, T, D), np.float32)
    for j in range(NCORES):
        o = results[j]["out"]  # [B, 2, 128, D]
        for b in range(B):
            for m in range(2):
                r0 = m * 1024 + j * P
                out[b, r0:r0 + P, :] = o[b, m]
    return out


def kernel(x, W_Q, W_K, W_V, W_O):
    x = np.asarray(x, np.float32)
    in_maps = make_in_maps(x, np.asarray(W_Q, np.float32),
                           np.asarray(W_K, np.float32),
                           np.asarray(W_V, np.float32),
                           np.asarray(W_O, np.float32))
    nc = _get_nc()
    res = run_bass_kernel_spmd(nc, in_maps, core_ids=list(range(NCORES)))
    return assemble(res.results)


# revision 43
# speedup vs baseline: 1.1563x; 1.0295x over previous
"""Causal multi-head attention (B=4, T=2048, D=1024, H=16) on 8 NeuronCores.

Sharding:
  stage 1 (QKV proj + attention): core c -> batch c//2, head-group c%2
    (8 of 16 heads, 512 of 1024 channels). Data-parallel on B, tensor-
    parallel on heads.
  stage 2 (output projection): one 8-rank AllToAll re-shards attention
    output to (all 4 batches x 256-token t-slice) per core, then each core
    computes out = attn_out @ W_O.T for its 1024 rows. No reduction needed.

Matmul operands are bf16 (same PE rate as fp32r at >=256 moving columns,
full rate below it, half the DMA/SBUF footprint); PSUM accumulation stays
fp32. exp runs on the scalar engine reading PSUM directly with the softmax
scale fused; the softmax denominator comes for free as a 65th output row of
the PV matmul (V augmented with a ones column). Causal masking multiplies
diagonal-block probabilities by precomputed 0/1 masks. The reciprocal
denominators are partition-broadcast by a small SBUF->SBUF DMA instead of a
PE ones-matmul.

Scheduling: attention is ACT-bound per k-tile (exp ~1040ns vs QK+PV
~850ns), so projection work for the NEXT chunk is queued and paced into the
attention stream as PE filler. The filler queue is labeled with
(chunk, kind, index) and force-drained at each consumer site (Q before the
attend that reads it, K/V before the k-tile that reads them), which lets the
pacer spread filler across the whole remaining schedule without correctness
risk. The m=0 output projection is deferred until after the second
collective is emitted so its PE work hides the collective+reshard latency
that gates the m=1 output projection.
"""
import numpy as np
import ml_dtypes

import concourse.bass as bass
import concourse.mybir as mybir
import concourse.tile as tile
from concourse.bass_utils import run_bass_kernel_spmd

F32 = mybir.dt.float32
F32R = mybir.dt.float32r
BF16 = mybir.dt.bfloat16

P = 128
B, T, D = 4, 2048, 1024
H, HD = 16, 64
NCORES = 8
CH = D // 2          # channels per core (8 heads)
NHP = 4              # head pairs per core
NKT = T // P         # 16 k-tiles
NQC = T // 512       # 4 q-chunks
NIT = D // P         # 8 input-dim tiles


def _split_multiwaits(nc) -> int:
    """walrus here rejects >1 sem wait per instruction; split extras into
    wait-only NoOps on the same engine."""
    nsplit = 0
    for f in nc.m.functions:
        for bb in f.blocks:
            if not any(
                i.sync_info is not None and i.sync_info.on_wait is not None
                and len(i.sync_info.on_wait) > 1 for i in bb.instructions
            ):
                continue
            new_list = []
            for inst in bb.instructions:
                si = inst.sync_info
                if si is not None and si.on_wait is not None and len(si.on_wait) > 1:
                    waits = list(si.on_wait)
                    for k, w in enumerate(waits[:-1]):
                        n = mybir.InstNoOp(
                            name=f"{inst.name}-wsplit{k}", ins=[], outs=[])
                        n.engine = inst.engine
                        n.sync_info = mybir.SyncInfo(on_wait=[w], on_update=[])
                        new_list.append(n)
                        nsplit += 1
                    inst.sync_info = mybir.SyncInfo(
                        on_wait=[waits[-1]], on_update=list(si.on_update or []))
                new_list.append(inst)
            bb.instructions = new_list
    return nsplit


def _build_nc(sim: bool = False):
    nc = bass.Bass("TRN2", target_bir_lowering=False, debug=False,
                   num_devices=NCORES)
    xt_d = nc.dram_tensor("xt", [D, T], BF16, kind="ExternalInput").ap()
    wq_d = nc.dram_tensor("wq", [D, CH], BF16, kind="ExternalInput").ap()
    wk_d = nc.dram_tensor("wk", [D, CH], BF16, kind="ExternalInput").ap()
    wv_d = nc.dram_tensor("wv", [D, CH], BF16, kind="ExternalInput").ap()
    wo_d = nc.dram_tensor("wo", [D, D], BF16, kind="ExternalInput").ap()
    out_d = nc.dram_tensor("out", [B, 2, P, D], F32, kind="ExternalOutput").ap()
    a2a_in0 = nc.dram_tensor("a2a_in0", [NCORES, CH, P], BF16).ap()
    a2a_out0 = nc.dram_tensor("a2a_out0", [NCORES, CH, P], BF16).ap()
    # the m=1 exchange is split by head-pair half: the hp0/1 half fires
    # mid-qc3 (its tails are done) so half the m=1 output projection is
    # available as late-qc3 filler; only the hp2/3 half gates the end
    a2a_in1a = nc.dram_tensor("a2a_in1a", [NCORES, CH // 2, P], BF16).ap()
    a2a_out1a = nc.dram_tensor("a2a_out1a", [NCORES, CH // 2, P], BF16).ap()
    a2a_in1b = nc.dram_tensor("a2a_in1b", [NCORES, CH // 2, P], BF16).ap()
    a2a_out1b = nc.dram_tensor("a2a_out1b", [NCORES, CH // 2, P], BF16).ap()

    scale = float(1.0 / np.sqrt(HD))
    # attention units (k-tiles) per q-chunk, for filler pacing
    UNITS = [4 * (qc + 1) * NHP for qc in range(NQC)]

    with tile.TileContext(nc) as tc:
        with (
            tc.tile_pool(name="persist", bufs=1) as persist,
        ):
            # ---- persistent SBUF tensors -------------------------------
            kt_s = persist.tile([P, NHP, T], BF16)    # K^T  (channels, k)
            va = persist.tile([P, NKT, NHP, 2, HD + 1], BF16)  # V | ones

            with (
                tc.tile_pool(name="wpool", bufs=1) as wpool,
                tc.tile_pool(name="xpool", bufs=2) as xpool,
                tc.tile_pool(name="aob_pool", bufs=8) as aob_pool,
                tc.tile_pool(name="osb_pool", bufs=6) as osb_pool,
                tc.tile_pool(name="qpool", bufs=2) as qpool,
                tc.tile_pool(name="ao_pool", bufs=2) as ao_pool,
                tc.tile_pool(name="mpool", bufs=1) as mpool,
                tc.tile_pool(name="pt_pool", bufs=4) as pt_pool,
                tc.tile_pool(name="nrm_pool", bufs=1) as nrm_pool,
                tc.tile_pool(name="ppool", bufs=2, space="PSUM") as ppool,
                tc.tile_pool(name="ps_s", bufs=2, space="PSUM") as ps_s,
                tc.tile_pool(name="ps_pv", bufs=1, space="PSUM") as ps_pv,
            ):
                wq = wpool.tile([P, NIT, CH], BF16)
                wk = wpool.tile([P, NIT, CH], BF16)
                wv = wpool.tile([P, NIT, CH], BF16)
                wo0 = wpool.tile([P, NIT, 512], BF16)
                wo1 = wpool.tile([P, NIT, 512], BF16)
                xt_r = xt_d.rearrange("(i p) t -> p i t", p=P)
                xtc0 = xpool.tile([P, NIT, 512], BF16, tag="xtc")
                # staged arrival: it=0 first (smallest useful unit), then
                # batched remainders — descriptor-heavy DMAs beat
                # instruction-overhead-bound ones (fixed HWDGE cost per
                # DMA), and x/wv interleave because the V matmuls for tile
                # `it` need both tensors' slices and the DMA device is
                # serial
                wv_r = wv_d.rearrange("(i p) o -> p i o", p=P)
                nc.sync.dma_start(xtc0[:, 0], xt_r[:, 0, 0:512])
                nc.sync.dma_start(wv[:, 0], wv_r[:, 0])
                nc.sync.dma_start(xtc0[:, 1:4], xt_r[:, 1:4, 0:512])
                nc.sync.dma_start(wv[:, 1:4], wv_r[:, 1:4])
                nc.sync.dma_start(xtc0[:, 4:], xt_r[:, 4:, 0:512])
                nc.sync.dma_start(wv[:, 4:], wv_r[:, 4:])
                nc.sync.dma_start(wq[:], wq_d.rearrange(
                    "(i p) o -> p i o", p=P))
                nc.sync.dma_start(wk[:], wk_d.rearrange(
                    "(i p) o -> p i o", p=P))

                # ones: stationary rows for the denominator broadcast
                # matmuls + broadcast-copy source for the V|ones column
                # (a strided bf16 memset into va fails the ISA memset
                # value-type check, so fill via ACT broadcast copy)
                ones64 = mpool.tile([P, 64], BF16, tag="ones64")
                nc.gpsimd.memset(ones64[:], 1.0)
                nc.scalar.copy(
                    va[:, :, :, :, HD],
                    ones64[:, 0:1].to_broadcast((P, NKT, NHP, 2)))
                # 0/1 causal mask for the 128-col diagonal block: in
                # block-local coords the visible region is q_local >= p for
                # every diagonal tile, so one tile serves all of them
                mask = mpool.tile([P, P], BF16, tag="mask")
                nc.gpsimd.memset(mask[:], 1.0)
                nc.gpsimd.affine_select(
                    out=mask[:], in_=mask[:],
                    compare_op=mybir.AluOpType.is_ge,
                    fill=0.0, base=0, channel_multiplier=-1,
                    pattern=[[1, P]])

                # pending projection psum-groups of upcoming chunks, emitted
                # as PE filler work inside the attention kt loops. Entries
                # are (label, fn) with label=(chunk, kind, idx) so consumer
                # sites can force-drain exactly what they depend on.
                pending = []    # (label, fn, cost_ns)
                normtail = []
                filler_acc = [0.0]
                pcost = [0.0]
                reserve = [0.0]  # ns of work kept for the post-collective gap

                def push(label, fn, cost):
                    pending.append((label, fn, cost))
                    pcost[0] += cost

                def pop_front():
                    lb, fn, cost = pending.pop(0)
                    pcost[0] -= cost
                    fn()

                def emit_fillers(remaining_units, boost=0.0):
                    # proportional pacing: spread the queue across the whole
                    # remaining schedule instead of draining it greedily
                    # (late ACT-bound units would idle PE). `boost` forces
                    # extra pops at known PE-stall sites.
                    if not pending or pcost[0] <= reserve[0]:
                        return
                    filler_acc[0] += boost + len(pending) / max(
                        1, remaining_units)
                    while (filler_acc[0] >= 1.0 and pending
                           and pcost[0] > reserve[0]):
                        filler_acc[0] -= 1.0
                        pop_front()

                def force_drain(chunk, kind, idx):
                    """Pop fillers (in order) until no queued entry matches
                    (chunk, kind, <=idx) — consumer is about to read them."""
                    while any(lb[0] == chunk and lb[1] == kind and lb[2] <= idx
                              for lb, _, _ in pending):
                        pop_front()

                def project(tc4, xtc=None):
                    """Queue QKV projection psum-groups for t-chunk tc4.
                    Returns the Q^T chunk tile; the groups themselves are
                    emitted later as PE filler inside attention."""
                    if xtc is None:
                        xtc = xpool.tile([P, NIT, 512], BF16, tag="xtc")
                        nc.sync.dma_start(
                            xtc[:], xt_r[:, :, tc4 * 512:(tc4 + 1) * 512])
                    qtc = qpool.tile([P, NHP, 512], BF16, tag="qtc")

                    def qk_group(w, dst, dsl, ot):
                        def g():
                            ps = ppool.tile([P, 512], F32, tag="proj")
                            for it in range(NIT):
                                nc.tensor.matmul(
                                    ps[:], w[:, it, ot * P:(ot + 1) * P],
                                    xtc[:, it], start=(it == 0),
                                    stop=(it == NIT - 1))
                            nc.vector.tensor_copy(dst[:, ot, dsl], ps[:])
                        return g

                    def v_group(tt4):
                        def g():
                            ps = ppool.tile([P, 512], F32, tag="proj")
                            for it in range(NIT):
                                nc.tensor.matmul(
                                    ps[:], xtc[:, it, tt4 * P:(tt4 + 1) * P],
                                    wv[:, it], start=(it == 0),
                                    stop=(it == NIT - 1))
                            nc.vector.tensor_copy(
                                va[:, tc4 * 4 + tt4, :, :, 0:HD],
                                ps[:].rearrange("p (hp h d) -> p hp h d",
                                                hp=NHP, h=2))
                        return g

                    if tc4 == 0:
                        # V first: wv+x arrive first and the four V groups
                        # run it-major across four concurrent psums, so each
                        # arriving (x, wv) DMA chunk feeds 4 matmuls instead
                        # of 1 during the DMA-bound startup ramp
                        def v_block0():
                            pss = [
                                ppool.tile([P, 512], F32, tag="proj",
                                           name="v0ps0"),
                                ppool.tile([P, 512], F32, tag="proj",
                                           name="v0ps1"),
                                ps_s.tile([P, 512], F32, tag="s2",
                                          name="v0ps2"),
                                ps_s.tile([P, 512], F32, tag="s2",
                                          name="v0ps3"),
                            ]
                            for it in range(NIT):
                                for tt4 in range(4):
                                    nc.tensor.matmul(
                                        pss[tt4][:],
                                        xtc[:, it, tt4 * P:(tt4 + 1) * P],
                                        wv[:, it], start=(it == 0),
                                        stop=(it == NIT - 1))
                            for tt4 in range(4):
                                nc.vector.tensor_copy(
                                    va[:, tt4, :, :, 0:HD],
                                    pss[tt4][:].rearrange(
                                        "p (hp h d) -> p hp h d",
                                        hp=NHP, h=2))
                        push((0, "v", 3), v_block0, 6816)
                        for ot in range(NHP):
                            push((0, "q", ot), qk_group(
                                wq, qtc, slice(0, 512), ot), 1706)
                            push((0, "k", ot), qk_group(
                                wk, kt_s,
                                slice(tc4 * 512, (tc4 + 1) * 512), ot), 1706)
                    else:
                        # interleave Q/K per head-pair so force-drain
                        # deadlines pop the minimum prefix
                        for ot in range(NHP):
                            push((tc4, "q", ot), qk_group(
                                wq, qtc, slice(0, 512), ot), 1706)
                            push((tc4, "k", ot), qk_group(
                                wk, kt_s,
                                slice(tc4 * 512, (tc4 + 1) * 512), ot), 1706)
                        for tt4 in range(4):
                            push((tc4, "v", tt4), v_group(tt4), 1706)
                    return qtc

                def attend(hp, qc, qtc, aoq, rem_after):
                    """Attention for head-pair hp, q-chunk qc. kt loop is
                    software-pipelined: QK(kt+1) issues before PV(kt) so PE
                    isn't stalled behind the exp of the current tile."""
                    nkt = 4 * (qc + 1)
                    force_drain(qc, "q", hp)
                    pva = ps_pv.tile([HD + 1, 512], F32, tag="pva")
                    pvb = ps_pv.tile([HD + 1, 512], F32, tag="pvb")
                    s2s = {}

                    def qk(kt):
                        force_drain(kt // 4, "k", hp)
                        ksl = slice(kt * P, (kt + 1) * P)
                        f0 = max(0, kt - 4 * qc) * P  # first visible q column
                        s2 = ps_s.tile([P, 1024], F32, tag="s2")
                        nc.tensor.matmul(s2[:, f0:512], kt_s[0:64, hp, ksl],
                                         qtc[0:64, hp, f0:],
                                         start=True, stop=True)
                        nc.tensor.matmul(s2[:, 512 + f0:1024],
                                         kt_s[64:128, hp, ksl],
                                         qtc[64:128, hp, f0:],
                                         start=True, stop=True)
                        s2s[kt] = s2

                    def softmax_pv(kt, remaining):
                        force_drain(kt // 4, "v", kt % 4)
                        s2 = s2s.pop(kt)
                        pt = pt_pool.tile([P, 2, 512], BF16, tag="pt")
                        di = kt - 4 * qc
                        # diagonal blocks: only columns >= f0 are causally
                        # visible; exp and PV restrict to them (kt==0 is
                        # always full-width, initializing every PSUM column
                        # of the PV accumulators). The diagonal 128-col
                        # block gets the additive -inf bias pre-exp.
                        f0 = max(0, di) * P
                        s2v = s2[:].rearrange("p (a b) -> p a b", a=2)
                        if f0 > 0:
                            nc.scalar.activation(
                                pt[:, :, f0:], s2v[:, :, f0:],
                                mybir.ActivationFunctionType.Exp,
                                scale=scale)
                        else:
                            nc.scalar.activation(
                                pt[:].rearrange("p a b -> p (a b)"), s2[:],
                                mybir.ActivationFunctionType.Exp,
                                scale=scale)
                        if di >= 0:
                            nc.vector.tensor_mul(
                                pt[:, :, f0:f0 + P], pt[:, :, f0:f0 + P],
                                mask[:, None, :].to_broadcast((P, 2, P)))
                        nc.tensor.matmul(pva[:, f0:], va[:, kt, hp, 0],
                                         pt[:, 0, f0:],
                                         start=(kt == 0), stop=(kt == nkt - 1))
                        nc.tensor.matmul(pvb[:, f0:], va[:, kt, hp, 1],
                                         pt[:, 1, f0:],
                                         start=(kt == 0), stop=(kt == nkt - 1))
                        if kt >= 2 and normtail:
                            normtail.pop(0)()
                        emit_fillers(remaining)

                    qk(0)
                    for kt in range(1, nkt):
                        qk(kt)
                        if kt == 1:
                            # cross-attend boundary: PV(0) waits on exp(0)
                            # and qk(2) on the s2 slot it frees — nothing
                            # attention-side can run, so force one filler in
                            emit_fillers(nkt - 1 + rem_after, boost=1.8)
                        softmax_pv(kt - 1, (nkt - kt) + rem_after)
                    softmax_pv(nkt - 1, 1 + rem_after)

                    final = hp == NHP - 1 and qc in (1, 3)
                    pvs = nrm_pool.tile([P, 2, 512], F32, tag="pvs")
                    rden = nrm_pool.tile([P, 2, 512], BF16, tag="rden")
                    if final:
                        # this tail gates a collective launch: shortest
                        # possible chain — reciprocals read the PSUM
                        # denominator rows directly and the copies split
                        # across DVE/ACT
                        with nc.allow_low_precision("f32r softmax denoms"):
                            nc.vector.reciprocal(rden[64:65, 0], pva[64:65])
                            nc.vector.reciprocal(rden[64:65, 1], pvb[64:65])
                        nc.vector.tensor_copy(pvs[0:65, 0], pva[:])
                        nc.scalar.copy(pvs[0:65, 1], pvb[:])
                        rba = ppool.tile([64, 512], F32, tag="proj")
                        rbb = ppool.tile([64, 512], F32, tag="proj")
                        nc.tensor.matmul(rba[:], ones64[64:65, :],
                                         rden[64:65, 0],
                                         start=True, stop=True)
                        nc.tensor.matmul(rbb[:], ones64[64:65, :],
                                         rden[64:65, 1],
                                         start=True, stop=True)
                        nc.vector.tensor_mul(aoq[0:64, hp], pvs[0:64, 0],
                                             rba[:])
                        nc.vector.tensor_mul(aoq[64:128, hp], pvs[0:64, 1],
                                             rbb[:])
                        dst = a2a_dst(qc, hp)
                        jsl = slice((qc % 2) * 4, (qc % 2) * 4 + 4)
                        nc.sync.dma_start(
                            dst[0:64, jsl],
                            aoq[0:64, hp].rearrange("p (j t) -> p j t", j=4))
                        nc.sync.dma_start(
                            dst[64:128, jsl],
                            aoq[64:128, hp].rearrange("p (j t) -> p j t",
                                                      j=4))
                        return
                    # copy PV accumulators out of PSUM fast (frees banks);
                    # defer the recip->broadcast->scale tail into the next
                    # head-pair's kt loop so PE never stalls behind it
                    nc.vector.tensor_copy(pvs[0:65, 0], pva[:])
                    nc.vector.tensor_copy(pvs[0:65, 1], pvb[:])
                    with nc.allow_low_precision("f32r softmax denominators"):
                        nc.vector.reciprocal(rden[64:65, 0], pvs[64:65, 0])
                        nc.vector.reciprocal(rden[64:65, 1], pvs[64:65, 1])

                    def tail(hp=hp, pvs=pvs, rden=rden):
                        rba = ppool.tile([64, 512], F32, tag="proj")
                        rbb = ppool.tile([64, 512], F32, tag="proj")
                        nc.tensor.matmul(rba[:], ones64[64:65, :],
                                         rden[64:65, 0],
                                         start=True, stop=True)
                        nc.tensor.matmul(rbb[:], ones64[64:65, :],
                                         rden[64:65, 1],
                                         start=True, stop=True)
                        nc.vector.tensor_mul(aoq[0:64, hp], pvs[0:64, 0],
                                             rba[:])
                        nc.vector.tensor_mul(aoq[64:128, hp], pvs[0:64, 1],
                                             rbb[:])
                        # ship this head-pair's slice to the exchange buffer
                        # immediately so the collective's inputs aren't gated
                        # on one bulk DMA burst at stage end
                        nc.sync.dma_start(
                            a2a_dst(qc, hp)[:, (qc % 2) * 4:(qc % 2) * 4 + 4],
                            aoq[:, hp].rearrange("p (j t) -> p j t", j=4))
                    normtail.append(tail)

                # interleaved: project chunk tc, then attention q-chunk tc,
                # streaming each finished chunk into the re-shard buffers.
                # stage-2 row owner of q = m*1024 + j*128 + p is core j, so
                # the first collective can fire once q < 1024 is done.
                a2a_r0 = a2a_in0.rearrange("j (hp p) t -> p hp j t", p=P)
                a2a_r1a = a2a_in1a.rearrange("j (hp p) t -> p hp j t", p=P)
                a2a_r1b = a2a_in1b.rearrange("j (hp p) t -> p hp j t", p=P)

                def a2a_dst(qc, hp):
                    if qc // 2 == 0:
                        return a2a_r0[:, hp]
                    r = a2a_r1a if hp < 2 else a2a_r1b
                    return r[:, hp % 2]

                def emit_collective(cin, cout):
                    if sim:
                        nc.sync.dma_start(cout, cin)
                    else:
                        nc.gpsimd.collective_compute(
                            "AllToAll", mybir.AluOpType.bypass,
                            replica_groups=[list(range(NCORES))],
                            ins=[cin], outs=[cout])

                def load_aob(b, cout, nct):
                    aob = aob_pool.tile([P, nct, P], BF16, tag="aob")
                    nc.sync.dma_start(
                        aob[:],
                        cout[2 * b:2 * b + 2].rearrange(
                            "s (c p) t -> p (s c) t", p=P))
                    return aob

                def o_group(b, m, aob, osb, w, oc):
                    ps = ppool.tile([P, 512], F32, tag="proj")
                    for ct in range(NIT):
                        nc.tensor.matmul(
                            ps[:], aob[:, ct], w[:, ct],
                            start=(ct == 0), stop=(ct == NIT - 1))
                    osl = slice(oc * 512, (oc + 1) * 512)
                    nc.vector.tensor_copy(osb[:, osl], ps[:])
                    if oc == 1:
                        nc.sync.dma_start(out_d[b, m], osb[:])

                def o_stage(b, m, aob_get):
                    """Two labeled filler entries (one per 512-col half of
                    W_O) sharing one staging tile."""
                    slot = {}

                    def g(oc):
                        def f():
                            if "osb" not in slot:
                                slot["osb"] = osb_pool.tile(
                                    [P, D], F32, tag="osb",
                                    name=f"osb{m}_{b}")
                            o_group(b, m, aob_get(b), slot["osb"],
                                    (wo0, wo1)[oc], oc)
                        return f
                    return [((9, "o", 2 * b + oc), g(oc), 1706)
                            for oc in range(2)]

                # m=1 output projection in two channel-halves: the hp0/1
                # contraction (W_O rows {0,1,4,5}) runs off collective 1a
                # as late-qc3 filler; the hp2/3 half accumulates on top
                # after collective 1b
                osb1 = {}
                HALF_A = (0, 1, 4, 5)
                HALF_B = (2, 3, 6, 7)

                def o_half(b, aob, oc, cts, first):
                    ps = ppool.tile([P, 512], F32, tag="proj")
                    for i, ct in enumerate(cts):
                        nc.tensor.matmul(
                            ps[:], aob[:, i], (wo0, wo1)[oc][:, ct],
                            start=(i == 0), stop=(i == len(cts) - 1))
                    osl = slice(oc * 512, (oc + 1) * 512)
                    if first:
                        nc.vector.tensor_copy(osb1[b][:, osl], ps[:])
                    else:
                        nc.vector.tensor_add(osb1[b][:, osl],
                                             osb1[b][:, osl], ps[:])
                        nc.sync.dma_start(out_d[b, 1, :, osl],
                                          osb1[b][:, osl])

                def o1a_stage(b, aob_get):
                    def g(oc):
                        def f():
                            if b not in osb1:
                                osb1[b] = osb_pool.tile(
                                    [P, D], F32, tag="osb",
                                    name=f"osb1_{b}")
                            o_half(b, aob_get(b), oc, HALF_A, True)
                        return f
                    return [((9, "oa", 2 * b + oc), g(oc), 852)
                            for oc in range(2)]

                aob0 = {}
                aob1a = {}
                aob1b = {}
                qtc = project(0, xtc=xtc0)
                # chunk 0 must fully project before attention starts
                while pending:
                    pop_front()
                for tc4 in range(NQC):
                    if tc4 + 1 < NQC:
                        next_qtc = project(tc4 + 1)  # queued as fillers
                    aoq = ao_pool.tile([P, NHP, 512], BF16, tag="aoq")
                    for hp in range(NHP):
                        if tc4 == 3 and hp == 2:
                            # hp0/1 tails of both m=1 chunks are shipped:
                            # fire the first m=1 half-collective and queue
                            # its output projection as late-qc3 filler
                            while normtail:
                                normtail.pop(0)()
                            emit_collective(a2a_in1a, a2a_out1a)
                            for b in range(B):
                                aob1a[b] = load_aob(b, a2a_out1a, 4)
                            for b in range(B):
                                for lb, fn, cost in o1a_stage(b, aob1a.get):
                                    push(lb, fn, cost)
                            reserve[0] = 12000
                        rem_after = (NHP - 1 - hp) * 4 * (tc4 + 1) + sum(
                            UNITS[tc4 + 1:])
                        attend(hp, tc4, qtc, aoq, rem_after)
                    if tc4 in (1, 3):
                        # the collective launch is gated on every tail DMA
                        # of its half; drain them now (fillers keep pacing)
                        while normtail:
                            normtail.pop(0)()
                    if tc4 == 1:
                        emit_collective(a2a_in0, a2a_out0)
                    if tc4 == 2:
                        # qc0/qc1 results finished resharding during qc2:
                        # W_O + the m=0 reshard tiles stream in now. The m=0
                        # output projection joins the filler queue behind
                        # chunk-3's projections, but at least 4 groups are
                        # held back (reserve) so PE has work after
                        # collective 1 is emitted, hiding the
                        # collective+reshard latency that gates m=1.
                        wo_r = wo_d.rearrange("(i p) o -> p i o", p=P)
                        nc.sync.dma_start(wo0[:], wo_r[:, :, 0:512])
                        nc.sync.dma_start(wo1[:], wo_r[:, :, 512:1024])
                        for b in range(B):
                            aob0[b] = load_aob(b, a2a_out0, NIT)
                        for b in range(B):
                            for lb, fn, cost in o_stage(b, 0, aob0.get):
                                push(lb, fn, cost)
                        reserve[0] = 12000
                    if tc4 + 1 < NQC:
                        qtc = next_qtc
                while pending and pcost[0] > reserve[0]:
                    pop_front()
                emit_collective(a2a_in1b, a2a_out1b)

                # m=1b reshard loads issue first: they only wait on the
                # collective, and queueing them behind other stores would
                # delay them on the in-order DMA queue
                for b in range(B):
                    aob1b[b] = load_aob(b, a2a_out1b, 4)

                # reserved groups hide the collective+reshard latency
                reserve[0] = 0.0
                while pending:
                    pop_front()

                # ---- m=1 hp2/3 half: accumulate + store ----------------
                for b in range(B):
                    for oc in range(2):
                        o_half(b, aob1b[b], oc, HALF_B, False)

    _split_multiwaits(nc)
    return nc


_NC_CACHE = None


def _get_nc():
    global _NC_CACHE
    if _NC_CACHE is None:
        _NC_CACHE = _build_nc()
    return _NC_CACHE


def make_in_maps(x, W_Q, W_K, W_V, W_O):
    bf = ml_dtypes.bfloat16
    wqt = np.ascontiguousarray(W_Q.T).astype(bf)
    wkt = np.ascontiguousarray(W_K.T).astype(bf)
    wvt = np.ascontiguousarray(W_V.T).astype(bf)
    wot = np.ascontiguousarray(W_O.T).astype(bf)
    in_maps = []
    for c in range(NCORES):
        b, g = c // 2, c % 2
        in_maps.append({
            "xt": np.ascontiguousarray(x[b].T).astype(bf),
            "wq": np.ascontiguousarray(wqt[:, g * CH:(g + 1) * CH]),
            "wk": np.ascontiguousarray(wkt[:, g * CH:(g + 1) * CH]),
            "wv": np.ascontiguousarray(wvt[:, g * CH:(g + 1) * CH]),
            "wo": wot,
        })
    return in_maps


def assemble(results):
    out = np.empty((B, T, D), np.float32)
    for j in range(NCORES):
        o = results[j]["out"]  # [B, 2, 128, D]
        for b in range(B):
            for m in range(2):
                r0 = m * 1024 + j * P
                out[b, r0:r0 + P, :] = o[b, m]
    return out


def kernel(x, W_Q, W_K, W_V, W_O):
    x = np.asarray(x, np.float32)
    in_maps = make_in_maps(x, np.asarray(W_Q, np.float32),
                           np.asarray(W_K, np.float32),
                           np.asarray(W_V, np.float32),
                           np.asarray(W_O, np.float32))
    nc = _get_nc()
    res = run_bass_kernel_spmd(nc, in_maps, core_ids=list(range(NCORES)))
    return assemble(res.results)


# revision 48
# speedup vs baseline: 1.1910x; 1.0300x over previous
"""Causal multi-head attention (B=4, T=2048, D=1024, H=16) on 8 NeuronCores.

Sharding:
  stage 1 (QKV proj + attention): core c -> batch c//2, head-group c%2
    (8 of 16 heads, 512 of 1024 channels). Data-parallel on B, tensor-
    parallel on heads.
  stage 2 (output projection): one 8-rank AllToAll re-shards attention
    output to (all 4 batches x 256-token t-slice) per core, then each core
    computes out = attn_out @ W_O.T for its 1024 rows. No reduction needed.

Matmul operands are bf16 (same PE rate as fp32r at >=256 moving columns,
full rate below it, half the DMA/SBUF footprint); PSUM accumulation stays
fp32. exp runs on the scalar engine reading PSUM directly with the softmax
scale fused; the softmax denominator comes for free as a 65th output row of
the PV matmul (V augmented with a ones column). Causal masking multiplies
diagonal-block probabilities by precomputed 0/1 masks. The reciprocal
denominators are partition-broadcast by a small SBUF->SBUF DMA instead of a
PE ones-matmul.

Scheduling: attention is ACT-bound per k-tile (exp ~1040ns vs QK+PV
~850ns), so projection work for the NEXT chunk is queued and paced into the
attention stream as PE filler. The filler queue is labeled with
(chunk, kind, index) and force-drained at each consumer site (Q before the
attend that reads it, K/V before the k-tile that reads them), which lets the
pacer spread filler across the whole remaining schedule without correctness
risk. The m=0 output projection is deferred until after the second
collective is emitted so its PE work hides the collective+reshard latency
that gates the m=1 output projection.
"""
import numpy as np
import ml_dtypes

import concourse.bass as bass
import concourse.mybir as mybir
import concourse.tile as tile
from concourse.bass_utils import run_bass_kernel_spmd

F32 = mybir.dt.float32
F32R = mybir.dt.float32r
BF16 = mybir.dt.bfloat16

P = 128
B, T, D = 4, 2048, 1024
H, HD = 16, 64
NCORES = 8
CH = D // 2          # channels per core (8 heads)
NHP = 4              # head pairs per core
NKT = T // P         # 16 k-tiles
NQC = T // 512       # 4 q-chunks
NIT = D // P         # 8 input-dim tiles


def _split_multiwaits(nc) -> int:
    """walrus here rejects >1 sem wait per instruction; split extras into
    wait-only NoOps on the same engine."""
    nsplit = 0
    for f in nc.m.functions:
        for bb in f.blocks:
            if not any(
                i.sync_info is not None and i.sync_info.on_wait is not None
                and len(i.sync_info.on_wait) > 1 for i in bb.instructions
            ):
                continue
            new_list = []
            for inst in bb.instructions:
                si = inst.sync_info
                if si is not None and si.on_wait is not None and len(si.on_wait) > 1:
                    waits = list(si.on_wait)
                    for k, w in enumerate(waits[:-1]):
                        n = mybir.InstNoOp(
                            name=f"{inst.name}-wsplit{k}", ins=[], outs=[])
                        n.engine = inst.engine
                        n.sync_info = mybir.SyncInfo(on_wait=[w], on_update=[])
                        new_list.append(n)
                        nsplit += 1
                    inst.sync_info = mybir.SyncInfo(
                        on_wait=[waits[-1]], on_update=list(si.on_update or []))
                new_list.append(inst)
            bb.instructions = new_list
    return nsplit


def _build_nc(sim: bool = False):
    nc = bass.Bass("TRN2", target_bir_lowering=False, debug=False,
                   num_devices=NCORES)
    xt_d = nc.dram_tensor("xt", [D, T], BF16, kind="ExternalInput").ap()
    wq_d = nc.dram_tensor("wq", [D, CH], BF16, kind="ExternalInput").ap()
    wk_d = nc.dram_tensor("wk", [D, CH], BF16, kind="ExternalInput").ap()
    wv_d = nc.dram_tensor("wv", [D, CH], BF16, kind="ExternalInput").ap()
    wo_d = nc.dram_tensor("wo", [D, D], BF16, kind="ExternalInput").ap()
    out_d = nc.dram_tensor("out", [B, 2, P, D], F32, kind="ExternalOutput").ap()
    a2a_in0 = nc.dram_tensor("a2a_in0", [NCORES, CH, P], BF16).ap()
    a2a_out0 = nc.dram_tensor("a2a_out0", [NCORES, CH, P], BF16).ap()
    # the m=1 exchange is split by head-pair half: the hp0/1 half fires
    # mid-qc3 (its tails are done) so half the m=1 output projection is
    # available as late-qc3 filler; only the hp2/3 half gates the end
    a2a_in1a = nc.dram_tensor("a2a_in1a", [NCORES, CH // 2, P], BF16).ap()
    a2a_out1a = nc.dram_tensor("a2a_out1a", [NCORES, CH // 2, P], BF16).ap()
    a2a_in1b = nc.dram_tensor("a2a_in1b", [NCORES, CH // 2, P], BF16).ap()
    a2a_out1b = nc.dram_tensor("a2a_out1b", [NCORES, CH // 2, P], BF16).ap()
    # bounce buffer for the softmax-reciprocal partition broadcast: DMA a
    # [1,2,512] row out and re-read it with a stride-0 partition source
    # (direct SBUF->SBUF partition broadcast is not expressible; engines
    # would need a ones-matmul, which costs PE rows on deferred tails)
    rden_d = nc.dram_tensor("rden_d", [8, 2, 512], BF16).ap()

    scale = float(1.0 / np.sqrt(HD))
    # attention units (k-tiles) per q-chunk, for filler pacing
    UNITS = [4 * (qc + 1) * NHP for qc in range(NQC)]

    with tile.TileContext(nc) as tc:
        with (
            tc.tile_pool(name="persist", bufs=1) as persist,
        ):
            # ---- persistent SBUF tensors -------------------------------
            kt_s = persist.tile([P, NHP, T], BF16)    # K^T  (channels, k)
            va = persist.tile([P, NKT, NHP, 2, HD + 1], BF16)  # V | ones

            with (
                tc.tile_pool(name="wpool", bufs=1) as wpool,
                tc.tile_pool(name="xpool", bufs=2) as xpool,
                tc.tile_pool(name="aob_pool", bufs=8) as aob_pool,
                tc.tile_pool(name="osb_pool", bufs=6) as osb_pool,
                tc.tile_pool(name="qpool", bufs=2) as qpool,
                tc.tile_pool(name="ao_pool", bufs=2) as ao_pool,
                tc.tile_pool(name="mpool", bufs=1) as mpool,
                tc.tile_pool(name="pt_pool", bufs=4) as pt_pool,
                tc.tile_pool(name="nrm_pool", bufs=1) as nrm_pool,
                tc.tile_pool(name="ppool", bufs=2, space="PSUM") as ppool,
                tc.tile_pool(name="ps_s", bufs=2, space="PSUM") as ps_s,
                tc.tile_pool(name="ps_pv", bufs=1, space="PSUM") as ps_pv,
            ):
                wq = wpool.tile([P, NIT, CH], BF16)
                wk = wpool.tile([P, NIT, CH], BF16)
                wv = wpool.tile([P, NIT, CH], BF16)
                wo0 = wpool.tile([P, NIT, 512], BF16)
                wo1 = wpool.tile([P, NIT, 512], BF16)
                xt_r = xt_d.rearrange("(i p) t -> p i t", p=P)
                xtc0 = xpool.tile([P, NIT, 512], BF16, tag="xtc")
                # staged arrival: it=0 first (smallest useful unit), then
                # batched remainders — descriptor-heavy DMAs beat
                # instruction-overhead-bound ones (fixed HWDGE cost per
                # DMA), and x/wv interleave because the V matmuls for tile
                # `it` need both tensors' slices and the DMA device is
                # serial
                wv_r = wv_d.rearrange("(i p) o -> p i o", p=P)
                nc.sync.dma_start(xtc0[:, 0], xt_r[:, 0, 0:512])
                nc.sync.dma_start(wv[:, 0], wv_r[:, 0])
                nc.sync.dma_start(xtc0[:, 1:4], xt_r[:, 1:4, 0:512])
                nc.sync.dma_start(wv[:, 1:4], wv_r[:, 1:4])
                nc.sync.dma_start(xtc0[:, 4:], xt_r[:, 4:, 0:512])
                nc.sync.dma_start(wv[:, 4:], wv_r[:, 4:])
                nc.sync.dma_start(wq[:], wq_d.rearrange(
                    "(i p) o -> p i o", p=P))
                nc.sync.dma_start(wk[:], wk_d.rearrange(
                    "(i p) o -> p i o", p=P))

                # ones: stationary rows for the denominator broadcast
                # matmuls + broadcast-copy source for the V|ones column
                # (a strided bf16 memset into va fails the ISA memset
                # value-type check, so fill via ACT broadcast copy)
                ones64 = mpool.tile([P, 64], BF16, tag="ones64")
                nc.gpsimd.memset(ones64[:], 1.0)
                nc.scalar.copy(
                    va[:, :, :, :, HD],
                    ones64[:, 0:1].to_broadcast((P, NKT, NHP, 2)))
                # 0/1 causal mask for the 128-col diagonal block: in
                # block-local coords the visible region is q_local >= p for
                # every diagonal tile, so one tile serves all of them
                mask = mpool.tile([P, P], BF16, tag="mask")
                nc.gpsimd.memset(mask[:], 1.0)
                nc.gpsimd.affine_select(
                    out=mask[:], in_=mask[:],
                    compare_op=mybir.AluOpType.is_ge,
                    fill=0.0, base=0, channel_multiplier=-1,
                    pattern=[[1, P]])

                # pending projection psum-groups of upcoming chunks, emitted
                # as PE filler work inside the attention kt loops. Entries
                # are (label, fn) with label=(chunk, kind, idx) so consumer
                # sites can force-drain exactly what they depend on.
                pending = []    # (label, fn, cost_ns)
                normtail = []
                filler_acc = [0.0]
                pcost = [0.0]
                reserve = [0.0]  # ns of work kept for the post-collective gap

                def push(label, fn, cost):
                    pending.append((label, fn, cost))
                    pcost[0] += cost

                def pop_front():
                    lb, fn, cost = pending.pop(0)
                    pcost[0] -= cost
                    fn()

                def emit_fillers(remaining_units, boost=0.0):
                    # proportional pacing: spread the queue across the whole
                    # remaining schedule instead of draining it greedily
                    # (late ACT-bound units would idle PE). `boost` forces
                    # extra pops at known PE-stall sites.
                    if not pending or pcost[0] <= reserve[0]:
                        return
                    filler_acc[0] += boost + len(pending) / max(
                        1, remaining_units)
                    while (filler_acc[0] >= 1.0 and pending
                           and pcost[0] > reserve[0]):
                        filler_acc[0] -= 1.0
                        pop_front()

                def force_drain(chunk, kind, idx):
                    """Pop fillers (in order) until no queued entry matches
                    (chunk, kind, <=idx) — consumer is about to read them."""
                    while any(lb[0] == chunk and lb[1] == kind and lb[2] <= idx
                              for lb, _, _ in pending):
                        pop_front()

                def project(tc4, xtc=None):
                    """Queue QKV projection psum-groups for t-chunk tc4.
                    Returns the Q^T chunk tile; the groups themselves are
                    emitted later as PE filler inside attention."""
                    if xtc is None:
                        xtc = xpool.tile([P, NIT, 512], BF16, tag="xtc")
                        nc.sync.dma_start(
                            xtc[:], xt_r[:, :, tc4 * 512:(tc4 + 1) * 512])
                    qtc = qpool.tile([P, NHP, 512], BF16, tag="qtc")

                    def qk_group(w, dst, dsl, ot):
                        def g():
                            ps = ppool.tile([P, 512], F32, tag="proj")
                            for it in range(NIT):
                                nc.tensor.matmul(
                                    ps[:], w[:, it, ot * P:(ot + 1) * P],
                                    xtc[:, it], start=(it == 0),
                                    stop=(it == NIT - 1))
                            nc.vector.tensor_copy(dst[:, ot, dsl], ps[:])
                        return g

                    def v_group(tt4):
                        def g():
                            ps = ppool.tile([P, 512], F32, tag="proj")
                            for it in range(NIT):
                                nc.tensor.matmul(
                                    ps[:], xtc[:, it, tt4 * P:(tt4 + 1) * P],
                                    wv[:, it], start=(it == 0),
                                    stop=(it == NIT - 1))
                            nc.vector.tensor_copy(
                                va[:, tc4 * 4 + tt4, :, :, 0:HD],
                                ps[:].rearrange("p (hp h d) -> p hp h d",
                                                hp=NHP, h=2))
                        return g

                    if tc4 == 0:
                        # V first: wv+x arrive first and the four V groups
                        # run it-major across four concurrent psums, so each
                        # arriving (x, wv) DMA chunk feeds 4 matmuls instead
                        # of 1 during the DMA-bound startup ramp
                        def v_block0():
                            pss = [
                                ppool.tile([P, 512], F32, tag="proj",
                                           name="v0ps0"),
                                ppool.tile([P, 512], F32, tag="proj",
                                           name="v0ps1"),
                                ps_s.tile([P, 512], F32, tag="s2",
                                          name="v0ps2"),
                                ps_s.tile([P, 512], F32, tag="s2",
                                          name="v0ps3"),
                            ]
                            for it in range(NIT):
                                for tt4 in range(4):
                                    nc.tensor.matmul(
                                        pss[tt4][:],
                                        xtc[:, it, tt4 * P:(tt4 + 1) * P],
                                        wv[:, it], start=(it == 0),
                                        stop=(it == NIT - 1))
                            for tt4 in range(4):
                                nc.vector.tensor_copy(
                                    va[:, tt4, :, :, 0:HD],
                                    pss[tt4][:].rearrange(
                                        "p (hp h d) -> p hp h d",
                                        hp=NHP, h=2))
                        push((0, "v", 3), v_block0, 6816)
                        for ot in range(NHP):
                            push((0, "q", ot), qk_group(
                                wq, qtc, slice(0, 512), ot), 1706)
                            push((0, "k", ot), qk_group(
                                wk, kt_s,
                                slice(tc4 * 512, (tc4 + 1) * 512), ot), 1706)
                    else:
                        # interleave Q/K per head-pair so force-drain
                        # deadlines pop the minimum prefix
                        for ot in range(NHP):
                            push((tc4, "q", ot), qk_group(
                                wq, qtc, slice(0, 512), ot), 1706)
                            push((tc4, "k", ot), qk_group(
                                wk, kt_s,
                                slice(tc4 * 512, (tc4 + 1) * 512), ot), 1706)
                        for tt4 in range(4):
                            push((tc4, "v", tt4), v_group(tt4), 1706)
                    return qtc

                def attend(hp, qc, qtc, aoq, rem_after):
                    """Attention for head-pair hp, q-chunk qc. kt loop is
                    software-pipelined: QK(kt+1) issues before PV(kt) so PE
                    isn't stalled behind the exp of the current tile."""
                    nkt = 4 * (qc + 1)
                    force_drain(qc, "q", hp)
                    pva = ps_pv.tile([HD + 1, 512], F32, tag="pva")
                    pvb = ps_pv.tile([HD + 1, 512], F32, tag="pvb")
                    s2s = {}

                    def qk(kt):
                        force_drain(kt // 4, "k", hp)
                        ksl = slice(kt * P, (kt + 1) * P)
                        f0 = max(0, kt - 4 * qc) * P  # first visible q column
                        s2 = ps_s.tile([P, 1024], F32, tag="s2")
                        nc.tensor.matmul(s2[:, f0:512], kt_s[0:64, hp, ksl],
                                         qtc[0:64, hp, f0:],
                                         start=True, stop=True)
                        nc.tensor.matmul(s2[:, 512 + f0:1024],
                                         kt_s[64:128, hp, ksl],
                                         qtc[64:128, hp, f0:],
                                         start=True, stop=True)
                        s2s[kt] = s2

                    def softmax_pv(kt, remaining):
                        force_drain(kt // 4, "v", kt % 4)
                        s2 = s2s.pop(kt)
                        pt = pt_pool.tile([P, 2, 512], BF16, tag="pt")
                        di = kt - 4 * qc
                        # diagonal blocks: only columns >= f0 are causally
                        # visible; exp and PV restrict to them (kt==0 is
                        # always full-width, initializing every PSUM column
                        # of the PV accumulators). The diagonal 128-col
                        # block gets the additive -inf bias pre-exp.
                        f0 = max(0, di) * P
                        s2v = s2[:].rearrange("p (a b) -> p a b", a=2)
                        if f0 > 0:
                            nc.scalar.activation(
                                pt[:, :, f0:], s2v[:, :, f0:],
                                mybir.ActivationFunctionType.Exp,
                                scale=scale)
                        else:
                            nc.scalar.activation(
                                pt[:].rearrange("p a b -> p (a b)"), s2[:],
                                mybir.ActivationFunctionType.Exp,
                                scale=scale)
                        if di >= 0:
                            nc.vector.tensor_mul(
                                pt[:, :, f0:f0 + P], pt[:, :, f0:f0 + P],
                                mask[:, None, :].to_broadcast((P, 2, P)))
                        nc.tensor.matmul(pva[:, f0:], va[:, kt, hp, 0],
                                         pt[:, 0, f0:],
                                         start=(kt == 0), stop=(kt == nkt - 1))
                        nc.tensor.matmul(pvb[:, f0:], va[:, kt, hp, 1],
                                         pt[:, 1, f0:],
                                         start=(kt == 0), stop=(kt == nkt - 1))
                        if kt >= 2 and normtail:
                            normtail.pop(0)()
                        emit_fillers(remaining)

                    qk(0)
                    for kt in range(1, nkt):
                        qk(kt)
                        if kt == 1:
                            # cross-attend boundary: PV(0) waits on exp(0)
                            # and qk(2) on the s2 slot it frees — nothing
                            # attention-side can run, so force one filler in
                            emit_fillers(nkt - 1 + rem_after, boost=1.8)
                        softmax_pv(kt - 1, (nkt - kt) + rem_after)
                    softmax_pv(nkt - 1, 1 + rem_after)

                    final = hp == NHP - 1 and qc in (1, 3)
                    pvs = nrm_pool.tile([P, 2, 512], F32, tag="pvs")
                    rden = nrm_pool.tile([P, 2, 512], BF16, tag="rden")
                    if final:
                        # this tail gates a collective launch: shortest
                        # possible chain — reciprocals read the PSUM
                        # denominator rows directly and the copies split
                        # across DVE/ACT
                        with nc.allow_low_precision("f32r softmax denoms"):
                            nc.vector.reciprocal(rden[64:65, 0], pva[64:65])
                            nc.vector.reciprocal(rden[64:65, 1], pvb[64:65])
                        nc.vector.tensor_copy(pvs[0:65, 0], pva[:])
                        nc.scalar.copy(pvs[0:65, 1], pvb[:])
                        rba = ppool.tile([64, 512], F32, tag="proj")
                        rbb = ppool.tile([64, 512], F32, tag="proj")
                        nc.tensor.matmul(rba[:], ones64[64:65, :],
                                         rden[64:65, 0],
                                         start=True, stop=True)
                        nc.tensor.matmul(rbb[:], ones64[64:65, :],
                                         rden[64:65, 1],
                                         start=True, stop=True)
                        nc.vector.tensor_mul(aoq[0:64, hp], pvs[0:64, 0],
                                             rba[:])
                        nc.vector.tensor_mul(aoq[64:128, hp], pvs[0:64, 1],
                                             rbb[:])
                        dst = a2a_dst(qc, hp)
                        jsl = slice((qc % 2) * 4, (qc % 2) * 4 + 4)
                        nc.sync.dma_start(
                            dst[0:64, jsl],
                            aoq[0:64, hp].rearrange("p (j t) -> p j t", j=4))
                        nc.sync.dma_start(
                            dst[64:128, jsl],
                            aoq[64:128, hp].rearrange("p (j t) -> p j t",
                                                      j=4))
                        return
                    # copy PV accumulators out of PSUM fast (frees banks);
                    # defer the recip->broadcast->scale tail into the next
                    # head-pair's kt loop so PE never stalls behind it
                    nc.vector.tensor_copy(pvs[0:65, 0], pva[:])
                    nc.vector.tensor_copy(pvs[0:65, 1], pvb[:])
                    with nc.allow_low_precision("f32r softmax denominators"):
                        nc.vector.reciprocal(rden[64:65, 0], pvs[64:65, 0])
                        nc.vector.reciprocal(rden[64:65, 1], pvs[64:65, 1])

                    def tail(hp=hp, qc=qc, pvs=pvs, rden=rden):
                        # partition-broadcast the reciprocals via a DRAM
                        # bounce (stride-0 source): slower than a
                        # ones-matmul but entirely off the PE, and deferred
                        # tails have an attend's worth of slack
                        sl = (qc * NHP + hp) % 8
                        rb = nrm_pool.tile([64, 2, 512], BF16, tag="rb")
                        nc.sync.dma_start(rden_d[sl], rden[64:65, :, :])
                        nc.sync.dma_start(
                            rb[:], rden_d[sl:sl + 1].to_broadcast(
                                (64, 2, 512)))
                        nc.vector.tensor_mul(aoq[0:64, hp], pvs[0:64, 0],
                                             rb[:, 0])
                        nc.vector.tensor_mul(aoq[64:128, hp], pvs[0:64, 1],
                                             rb[:, 1])
                        # ship this head-pair's slice to the exchange buffer
                        # immediately so the collective's inputs aren't gated
                        # on one bulk DMA burst at stage end
                        nc.sync.dma_start(
                            a2a_dst(qc, hp)[:, (qc % 2) * 4:(qc % 2) * 4 + 4],
                            aoq[:, hp].rearrange("p (j t) -> p j t", j=4))
                    normtail.append(tail)

                # interleaved: project chunk tc, then attention q-chunk tc,
                # streaming each finished chunk into the re-shard buffers.
                # stage-2 row owner of q = m*1024 + j*128 + p is core j, so
                # the first collective can fire once q < 1024 is done.
                a2a_r0 = a2a_in0.rearrange("j (hp p) t -> p hp j t", p=P)
                a2a_r1a = a2a_in1a.rearrange("j (hp p) t -> p hp j t", p=P)
                a2a_r1b = a2a_in1b.rearrange("j (hp p) t -> p hp j t", p=P)

                def a2a_dst(qc, hp):
                    if qc // 2 == 0:
                        return a2a_r0[:, hp]
                    r = a2a_r1a if hp < 2 else a2a_r1b
                    return r[:, hp % 2]

                def emit_collective(cin, cout):
                    if sim:
                        nc.sync.dma_start(cout, cin)
                    else:
                        nc.gpsimd.collective_compute(
                            "AllToAll", mybir.AluOpType.bypass,
                            replica_groups=[list(range(NCORES))],
                            ins=[cin], outs=[cout])

                def load_aob(b, cout, nct):
                    aob = aob_pool.tile([P, nct, P], BF16, tag="aob")
                    nc.sync.dma_start(
                        aob[:],
                        cout[2 * b:2 * b + 2].rearrange(
                            "s (c p) t -> p (s c) t", p=P))
                    return aob

                def o_group(b, m, aob, osb, w, oc):
                    ps = ppool.tile([P, 512], F32, tag="proj")
                    for ct in range(NIT):
                        nc.tensor.matmul(
                            ps[:], aob[:, ct], w[:, ct],
                            start=(ct == 0), stop=(ct == NIT - 1))
                    osl = slice(oc * 512, (oc + 1) * 512)
                    nc.vector.tensor_copy(osb[:, osl], ps[:])
                    if oc == 1:
                        nc.sync.dma_start(out_d[b, m], osb[:])

                def o_stage(b, m, aob_get):
                    """Two labeled filler entries (one per 512-col half of
                    W_O) sharing one staging tile."""
                    slot = {}

                    def g(oc):
                        def f():
                            if "osb" not in slot:
                                slot["osb"] = osb_pool.tile(
                                    [P, D], F32, tag="osb",
                                    name=f"osb{m}_{b}")
                            o_group(b, m, aob_get(b), slot["osb"],
                                    (wo0, wo1)[oc], oc)
                        return f
                    return [((9, "o", 2 * b + oc), g(oc), 1706)
                            for oc in range(2)]

                # m=1 output projection in two channel-halves: the hp0/1
                # contraction (W_O rows {0,1,4,5}) runs off collective 1a
                # as late-qc3 filler; the hp2/3 half accumulates on top
                # after collective 1b
                osb1 = {}
                HALF_A = (0, 1, 4, 5)
                HALF_B = (2, 3, 6, 7)

                def o_half(b, aob, oc, cts, first):
                    ps = ppool.tile([P, 512], F32, tag="proj")
                    for i, ct in enumerate(cts):
                        nc.tensor.matmul(
                            ps[:], aob[:, i], (wo0, wo1)[oc][:, ct],
                            start=(i == 0), stop=(i == len(cts) - 1))
                    osl = slice(oc * 512, (oc + 1) * 512)
                    if first:
                        nc.vector.tensor_copy(osb1[b][:, osl], ps[:])
                    else:
                        nc.vector.tensor_add(osb1[b][:, osl],
                                             osb1[b][:, osl], ps[:])
                        nc.sync.dma_start(out_d[b, 1, :, osl],
                                          osb1[b][:, osl])

                def o1a_stage(b, aob_get):
                    def g(oc):
                        def f():
                            if b not in osb1:
                                osb1[b] = osb_pool.tile(
                                    [P, D], F32, tag="osb",
                                    name=f"osb1_{b}")
                            o_half(b, aob_get(b), oc, HALF_A, True)
                        return f
                    return [((9, "oa", 2 * b + oc), g(oc), 852)
                            for oc in range(2)]

                aob0 = {}
                aob1a = {}
                aob1b = {}
                qtc = project(0, xtc=xtc0)
                # chunk 0 must fully project before attention starts
                while pending:
                    pop_front()
                for tc4 in range(NQC):
                    if tc4 + 1 < NQC:
                        next_qtc = project(tc4 + 1)  # queued as fillers
                    aoq = ao_pool.tile([P, NHP, 512], BF16, tag="aoq")
                    for hp in range(NHP):
                        if tc4 == 3 and hp == 2:
                            # hp0/1 tails of both m=1 chunks are shipped:
                            # fire the first m=1 half-collective and queue
                            # its output projection as late-qc3 filler
                            while normtail:
                                normtail.pop(0)()
                            emit_collective(a2a_in1a, a2a_out1a)
                            for b in range(B):
                                aob1a[b] = load_aob(b, a2a_out1a, 4)
                            for b in range(B):
                                for lb, fn, cost in o1a_stage(b, aob1a.get):
                                    push(lb, fn, cost)
                            reserve[0] = 12000
                        rem_after = (NHP - 1 - hp) * 4 * (tc4 + 1) + sum(
                            UNITS[tc4 + 1:])
                        attend(hp, tc4, qtc, aoq, rem_after)
                    if tc4 in (1, 3):
                        # the collective launch is gated on every tail DMA
                        # of its half; drain them now (fillers keep pacing)
                        while normtail:
                            normtail.pop(0)()
                    if tc4 == 1:
                        emit_collective(a2a_in0, a2a_out0)
                    if tc4 == 2:
                        # qc0/qc1 results finished resharding during qc2:
                        # W_O + the m=0 reshard tiles stream in now. The m=0
                        # output projection joins the filler queue behind
                        # chunk-3's projections, but at least 4 groups are
                        # held back (reserve) so PE has work after
                        # collective 1 is emitted, hiding the
                        # collective+reshard latency that gates m=1.
                        wo_r = wo_d.rearrange("(i p) o -> p i o", p=P)
                        nc.sync.dma_start(wo0[:], wo_r[:, :, 0:512])
                        nc.sync.dma_start(wo1[:], wo_r[:, :, 512:1024])
                        for b in range(B):
                            aob0[b] = load_aob(b, a2a_out0, NIT)
                        for b in range(B):
                            for lb, fn, cost in o_stage(b, 0, aob0.get):
                                push(lb, fn, cost)
                        reserve[0] = 12000
                    if tc4 + 1 < NQC:
                        qtc = next_qtc
                while pending and pcost[0] > reserve[0]:
                    pop_front()
                emit_collective(a2a_in1b, a2a_out1b)

                # m=1b reshard loads issue first: they only wait on the
                # collective, and queueing them behind other stores would
                # delay them on the in-order DMA queue
                for b in range(B):
                    aob1b[b] = load_aob(b, a2a_out1b, 4)

                # reserved groups hide the collective+reshard latency
                reserve[0] = 0.0
                while pending:
                    pop_front()

                # ---- m=1 hp2/3 half: accumulate + store ----------------
                for b in range(B):
                    for oc in range(2):
                        o_half(b, aob1b[b], oc, HALF_B, False)

    _split_multiwaits(nc)
    return nc


_NC_CACHE = None


def _get_nc():
    global _NC_CACHE
    if _NC_CACHE is None:
        _NC_CACHE = _build_nc()
    return _NC_CACHE


def make_in_maps(x, W_Q, W_K, W_V, W_O):
    bf = ml_dtypes.bfloat16
    wqt = np.ascontiguousarray(W_Q.T).astype(bf)
    wkt = np.ascontiguousarray(W_K.T).astype(bf)
    wvt = np.ascontiguousarray(W_V.T).astype(bf)
    wot = np.ascontiguousarray(W_O.T).astype(bf)
    in_maps = []
    for c in range(NCORES):
        b, g = c // 2, c % 2
        in_maps.append({
            "xt": np.ascontiguousarray(x[b].T).astype(bf),
            "wq": np.ascontiguousarray(wqt[:, g * CH:(g + 1) * CH]),
            "wk": np.ascontiguousarray(wkt[:, g * CH:(g + 1) * CH]),
            "wv": np.ascontiguousarray(wvt[:, g * CH:(g + 1) * CH]),
            "wo": wot,
        })
    return in_maps


def assemble(results):
    out = np.empty((B, T, D), np.float32)
    for j in range(NCORES):
        o = results[j]["out"]  # [B, 2, 128, D]
        for b in range(B):
            for m in range(2):
                r0 = m * 1024 + j * P
                out[b, r0:r0 + P, :] = o[b, m]
    return out


def kernel(x, W_Q, W_K, W_V, W_O):
    x = np.asarray(x, np.float32)
    in_maps = make_in_maps(x, np.asarray(W_Q, np.float32),
                           np.asarray(W_K, np.float32),
                           np.asarray(W_V, np.float32),
                           np.asarray(W_O, np.float32))
    nc = _get_nc()
    res = run_bass_kernel_spmd(nc, in_maps, core_ids=list(range(NCORES)))
    return assemble(res.results)


# revision 68
# speedup vs baseline: 1.2084x; 1.0146x over previous
"""Causal multi-head attention (B=4, T=2048, D=1024, H=16) on 8 NeuronCores.

Sharding:
  stage 1 (QKV proj + attention): core c -> batch c//2, head-group c%2
    (8 of 16 heads, 512 of 1024 channels). Data-parallel on B, tensor-
    parallel on heads.
  stage 2 (output projection): one 8-rank AllToAll re-shards attention
    output to (all 4 batches x 256-token t-slice) per core, then each core
    computes out = attn_out @ W_O.T for its 1024 rows. No reduction needed.

Matmul operands are bf16 (same PE rate as fp32r at >=256 moving columns,
full rate below it, half the DMA/SBUF footprint); PSUM accumulation stays
fp32. exp runs on the scalar engine reading PSUM directly with the softmax
scale fused; the softmax denominator comes for free as a 65th output row of
the PV matmul (V augmented with a ones column). Causal masking multiplies
the 128-col diagonal block by a single block-local 0/1 mask. Softmax
reciprocals are partition-broadcast through a DRAM bounce buffer on
deferred tails (zero PE cost) and by ones-matmuls on the two
collective-gating tails (lowest latency).

Scheduling: attention is ACT-bound per k-tile (exp ~1040ns vs QK+PV
~850ns), so projection work for the NEXT chunk is queued and paced into the
attention stream as PE filler. The filler queue is labeled with
(chunk, kind, index) and force-drained at each consumer site (Q before the
attend that reads it, K/V before the k-tile that reads them), which lets
the pacer spread filler across the whole remaining schedule without
correctness risk; a fixed-boost pop at each attend boundary covers the
software-pipeline refill bubble. Large input loads are batched into few
descriptor-heavy DMAs (HWDGE charges a fixed ~625ns per DMA instruction).
The endgame keeps PE fed through both collectives: the m=0 output
projection and the hp0/1 half of the m=1 projection (split AllToAll) run
as late-qc3 + post-collective filler, with ~12us of queue held back to
cover the final collective+reshard latency that gates the m=1 hp2/3 half.
"""
import numpy as np
import ml_dtypes

import concourse.bass as bass
import concourse.mybir as mybir
import concourse.tile as tile
from concourse.bass_utils import run_bass_kernel_spmd

F32 = mybir.dt.float32
F32R = mybir.dt.float32r
BF16 = mybir.dt.bfloat16

P = 128
B, T, D = 4, 2048, 1024
H, HD = 16, 64
NCORES = 8
CH = D // 2          # channels per core (8 heads)
NHP = 4              # head pairs per core
NKT = T // P         # 16 k-tiles
NQC = T // 512       # 4 q-chunks
NIT = D // P         # 8 input-dim tiles


def _split_multiwaits(nc) -> int:
    """walrus here rejects >1 sem wait per instruction; split extras into
    wait-only NoOps on the same engine."""
    nsplit = 0
    for f in nc.m.functions:
        for bb in f.blocks:
            if not any(
                i.sync_info is not None and i.sync_info.on_wait is not None
                and len(i.sync_info.on_wait) > 1 for i in bb.instructions
            ):
                continue
            new_list = []
            for inst in bb.instructions:
                si = inst.sync_info
                if si is not None and si.on_wait is not None and len(si.on_wait) > 1:
                    waits = list(si.on_wait)
                    for k, w in enumerate(waits[:-1]):
                        n = mybir.InstNoOp(
                            name=f"{inst.name}-wsplit{k}", ins=[], outs=[])
                        n.engine = inst.engine
                        n.sync_info = mybir.SyncInfo(on_wait=[w], on_update=[])
                        new_list.append(n)
                        nsplit += 1
                    inst.sync_info = mybir.SyncInfo(
                        on_wait=[waits[-1]], on_update=list(si.on_update or []))
                new_list.append(inst)
            bb.instructions = new_list
    return nsplit


def _build_nc(sim: bool = False):
    nc = bass.Bass("TRN2", target_bir_lowering=False, debug=False,
                   num_devices=NCORES)
    xt_d = nc.dram_tensor("xt", [D, T], BF16, kind="ExternalInput").ap()
    wq_d = nc.dram_tensor("wq", [D, CH], BF16, kind="ExternalInput").ap()
    wk_d = nc.dram_tensor("wk", [D, CH], BF16, kind="ExternalInput").ap()
    wv_d = nc.dram_tensor("wv", [D, CH], BF16, kind="ExternalInput").ap()
    wo_d = nc.dram_tensor("wo", [D, D], BF16, kind="ExternalInput").ap()
    out_d = nc.dram_tensor("out", [B, 2, P, D], F32, kind="ExternalOutput").ap()
    a2a_in0 = nc.dram_tensor("a2a_in0", [NCORES, CH, P], BF16).ap()
    a2a_out0 = nc.dram_tensor("a2a_out0", [NCORES, CH, P], BF16).ap()
    # the m=1 exchange is split by head-pair half: the hp0/1 half fires
    # mid-qc3 (its tails are done) so half the m=1 output projection is
    # available as late-qc3 filler; only the hp2/3 half gates the end
    a2a_in1a = nc.dram_tensor("a2a_in1a", [NCORES, CH // 2, P], BF16).ap()
    a2a_out1a = nc.dram_tensor("a2a_out1a", [NCORES, CH // 2, P], BF16).ap()
    a2a_in1b = nc.dram_tensor("a2a_in1b", [NCORES, CH // 2, P], BF16).ap()
    a2a_out1b = nc.dram_tensor("a2a_out1b", [NCORES, CH // 2, P], BF16).ap()
    # bounce buffer for the softmax-reciprocal partition broadcast: DMA a
    # [1,2,512] row out and re-read it with a stride-0 partition source
    # (direct SBUF->SBUF partition broadcast is not expressible; engines
    # would need a ones-matmul, which costs PE rows on deferred tails)
    rden_d = nc.dram_tensor("rden_d", [8, 2, 512], BF16).ap()

    scale = float(1.0 / np.sqrt(HD))
    # attention units (k-tiles) per q-chunk, for filler pacing
    UNITS = [4 * (qc + 1) * NHP for qc in range(NQC)]

    with tile.TileContext(nc) as tc:
        with (
            tc.tile_pool(name="persist", bufs=1) as persist,
        ):
            # ---- persistent SBUF tensors -------------------------------
            kt_s = persist.tile([P, NHP, T], BF16)    # K^T  (channels, k)
            va = persist.tile([P, NKT, NHP, 2, HD + 1], BF16)  # V | ones

            with (
                tc.tile_pool(name="wpool", bufs=1) as wpool,
                tc.tile_pool(name="xpool", bufs=2) as xpool,
                tc.tile_pool(name="aob_pool", bufs=8) as aob_pool,
                tc.tile_pool(name="osb_pool", bufs=6) as osb_pool,
                tc.tile_pool(name="qpool", bufs=2) as qpool,
                tc.tile_pool(name="ao_pool", bufs=3) as ao_pool,
                tc.tile_pool(name="mpool", bufs=1) as mpool,
                tc.tile_pool(name="pt_pool", bufs=6) as pt_pool,
                tc.tile_pool(name="nrm_pool", bufs=2) as nrm_pool,
                tc.tile_pool(name="ppool", bufs=2, space="PSUM") as ppool,
                tc.tile_pool(name="ps_s", bufs=2, space="PSUM") as ps_s,
                tc.tile_pool(name="ps_pv", bufs=1, space="PSUM") as ps_pv,
            ):
                wq = wpool.tile([P, NIT, CH], BF16)
                wk = wpool.tile([P, NIT, CH], BF16)
                wv = wpool.tile([P, NIT, CH], BF16)
                wo0 = wpool.tile([P, NIT, 512], BF16)
                wo1 = wpool.tile([P, NIT, 512], BF16)
                xt_r = xt_d.rearrange("(i p) t -> p i t", p=P)
                xtc0 = xpool.tile([P, NIT, 512], BF16, tag="xtc")
                # staged arrival: it=0 first (smallest useful unit), then
                # batched remainders — descriptor-heavy DMAs beat
                # instruction-overhead-bound ones (fixed HWDGE cost per
                # DMA), and x/wv interleave because the V matmuls for tile
                # `it` need both tensors' slices and the DMA device is
                # serial
                wv_r = wv_d.rearrange("(i p) o -> p i o", p=P)
                nc.sync.dma_start(xtc0[:, 0], xt_r[:, 0, 0:512])
                nc.sync.dma_start(wv[:, 0], wv_r[:, 0])
                nc.sync.dma_start(xtc0[:, 1:4], xt_r[:, 1:4, 0:512])
                nc.sync.dma_start(wv[:, 1:4], wv_r[:, 1:4])
                nc.sync.dma_start(xtc0[:, 4:], xt_r[:, 4:, 0:512])
                nc.sync.dma_start(wv[:, 4:], wv_r[:, 4:])
                nc.sync.dma_start(wq[:], wq_d.rearrange(
                    "(i p) o -> p i o", p=P))
                nc.sync.dma_start(wk[:], wk_d.rearrange(
                    "(i p) o -> p i o", p=P))

                # ones: stationary rows for the denominator broadcast
                # matmuls + broadcast-copy source for the V|ones column
                # (a strided bf16 memset into va fails the ISA memset
                # value-type check, so fill via ACT broadcast copy)
                ones64 = mpool.tile([P, 64], BF16, tag="ones64")
                nc.gpsimd.memset(ones64[:], 1.0)

                # p-state warmup: the tensor engine runs at half clock until
                # it has been continuously busy for 3us. Burn that ramp on
                # dummy matmuls during the initial DMA wait so the real
                # projections start at full speed.
                warm = mpool.tile([1, 512], BF16, tag="warm")
                nc.vector.memset(warm[:], 1.0)
                wps = ppool.tile([64, 512], F32, tag="proj")
                for _ in range(12):
                    nc.tensor.matmul(wps[:, 0:128], warm[0:1, 0:64],
                                     warm[0:1, 0:128],
                                     start=True, stop=True)
                nc.scalar.copy(
                    va[:, :, :, :, HD],
                    ones64[:, 0:1].to_broadcast((P, NKT, NHP, 2)))
                # 0/1 causal mask for the 128-col diagonal block: in
                # block-local coords the visible region is q_local >= p for
                # every diagonal tile, so one tile serves all of them
                mask = mpool.tile([P, P], BF16, tag="mask")
                nc.gpsimd.memset(mask[:], 1.0)
                nc.gpsimd.affine_select(
                    out=mask[:], in_=mask[:],
                    compare_op=mybir.AluOpType.is_ge,
                    fill=0.0, base=0, channel_multiplier=-1,
                    pattern=[[1, P]])

                # pending projection psum-groups of upcoming chunks, emitted
                # as PE filler work inside the attention kt loops. Entries
                # are (label, fn) with label=(chunk, kind, idx) so consumer
                # sites can force-drain exactly what they depend on.
                pending = []    # (label, fn, cost_ns)
                normtail = []
                filler_acc = [0.0]
                pcost = [0.0]
                reserve = [0.0]  # ns of work kept for the post-collective gap

                def push(label, fn, cost):
                    pending.append((label, fn, cost))
                    pcost[0] += cost

                def pop_front():
                    lb, fn, cost = pending.pop(0)
                    pcost[0] -= cost
                    fn()

                def emit_fillers(remaining_units, boost=0.0):
                    # proportional pacing: spread the queue across the whole
                    # remaining schedule instead of draining it greedily
                    # (late ACT-bound units would idle PE). `boost` forces
                    # extra pops at known PE-stall sites.
                    if not pending or pcost[0] <= reserve[0]:
                        return
                    filler_acc[0] += boost + len(pending) / max(
                        1, remaining_units)
                    while (filler_acc[0] >= 1.0 and pending
                           and pcost[0] > reserve[0]):
                        filler_acc[0] -= 1.0
                        pop_front()

                def force_drain(chunk, kind, idx):
                    """Pop fillers (in order) until no queued entry matches
                    (chunk, kind, <=idx) — consumer is about to read them."""
                    while any(lb[0] == chunk and lb[1] == kind and lb[2] <= idx
                              for lb, _, _ in pending):
                        pop_front()

                def project(tc4, xtc=None):
                    """Queue QKV projection psum-groups for t-chunk tc4.
                    Returns the Q^T chunk tile; the groups themselves are
                    emitted later as PE filler inside attention."""
                    if xtc is None:
                        xtc = xpool.tile([P, NIT, 512], BF16, tag="xtc")
                        nc.sync.dma_start(
                            xtc[:], xt_r[:, :, tc4 * 512:(tc4 + 1) * 512])
                    qtc = qpool.tile([P, NHP, 512], BF16, tag="qtc")

                    def qk_group(w, dst, dsl, ot):
                        # two half-contraction pieces sharing one psum tile:
                        # finer filler quanta track the per-k-tile PE deficit
                        # during ACT-bound attention much more closely
                        st = {}

                        def ga():
                            st["ps"] = ppool.tile([P, 512], F32, tag="proj",
                                                  name=f"qk{tc4}_{ot}")
                            for it in range(NIT // 2):
                                nc.tensor.matmul(
                                    st["ps"][:], w[:, it, ot * P:(ot + 1) * P],
                                    xtc[:, it], start=(it == 0), stop=False)

                        def gb():
                            ps = st["ps"]
                            for it in range(NIT // 2, NIT):
                                nc.tensor.matmul(
                                    ps[:], w[:, it, ot * P:(ot + 1) * P],
                                    xtc[:, it], start=False,
                                    stop=(it == NIT - 1))
                            nc.vector.tensor_copy(dst[:, ot, dsl], ps[:])
                        return ga, gb

                    def v_group(tt4):
                        st = {}

                        def ga():
                            st["ps"] = ppool.tile([P, 512], F32, tag="proj",
                                                  name=f"v{tc4}_{tt4}")
                            for it in range(NIT // 2):
                                nc.tensor.matmul(
                                    st["ps"][:],
                                    xtc[:, it, tt4 * P:(tt4 + 1) * P],
                                    wv[:, it], start=(it == 0), stop=False)

                        def gb():
                            ps = st["ps"]
                            for it in range(NIT // 2, NIT):
                                nc.tensor.matmul(
                                    ps[:], xtc[:, it, tt4 * P:(tt4 + 1) * P],
                                    wv[:, it], start=False,
                                    stop=(it == NIT - 1))
                            nc.vector.tensor_copy(
                                va[:, tc4 * 4 + tt4, :, :, 0:HD],
                                ps[:].rearrange("p (hp h d) -> p hp h d",
                                                hp=NHP, h=2))
                        return ga, gb

                    if tc4 == 0:
                        # V first: wv+x arrive first and the four V groups
                        # run it-major across four concurrent psums, so each
                        # arriving (x, wv) DMA chunk feeds 4 matmuls instead
                        # of 1 during the DMA-bound startup ramp
                        def v_block0():
                            pss = [
                                ppool.tile([P, 512], F32, tag="proj",
                                           name="v0ps0"),
                                ppool.tile([P, 512], F32, tag="proj",
                                           name="v0ps1"),
                                ps_s.tile([P, 512], F32, tag="s2",
                                          name="v0ps2"),
                                ps_s.tile([P, 512], F32, tag="s2",
                                          name="v0ps3"),
                            ]
                            for it in range(NIT):
                                for tt4 in range(4):
                                    nc.tensor.matmul(
                                        pss[tt4][:],
                                        xtc[:, it, tt4 * P:(tt4 + 1) * P],
                                        wv[:, it], start=(it == 0),
                                        stop=(it == NIT - 1))
                            for tt4 in range(4):
                                nc.vector.tensor_copy(
                                    va[:, tt4, :, :, 0:HD],
                                    pss[tt4][:].rearrange(
                                        "p (hp h d) -> p hp h d",
                                        hp=NHP, h=2))
                        push((0, "v", 3), v_block0, 6816)
                        for ot in range(NHP):
                            for g in qk_group(wq, qtc, slice(0, 512), ot):
                                push((0, "q", ot), g, 853)
                            for g in qk_group(
                                    wk, kt_s,
                                    slice(tc4 * 512, (tc4 + 1) * 512), ot):
                                push((0, "k", ot), g, 853)
                    else:
                        # interleave Q/K per head-pair so force-drain
                        # deadlines pop the minimum prefix
                        for ot in range(NHP):
                            for g in qk_group(wq, qtc, slice(0, 512), ot):
                                push((tc4, "q", ot), g, 853)
                            for g in qk_group(
                                    wk, kt_s,
                                    slice(tc4 * 512, (tc4 + 1) * 512), ot):
                                push((tc4, "k", ot), g, 853)
                        for tt4 in range(4):
                            for g in v_group(tt4):
                                push((tc4, "v", tt4), g, 853)
                    return qtc

                def attend(hp, qc, qtc, aoq, rem_after):
                    """Attention for head-pair hp, q-chunk qc. kt loop is
                    software-pipelined: QK(kt+1) issues before PV(kt) so PE
                    isn't stalled behind the exp of the current tile."""
                    nkt = 4 * (qc + 1)
                    force_drain(qc, "q", hp)
                    pva = ps_pv.tile([HD + 1, 512], F32, tag="pva")
                    pvb = ps_pv.tile([HD + 1, 512], F32, tag="pvb")
                    s2s = {}

                    def qk(kt):
                        force_drain(kt // 4, "k", hp)
                        ksl = slice(kt * P, (kt + 1) * P)
                        f0 = max(0, kt - 4 * qc) * P  # first visible q column
                        s2 = ps_s.tile([P, 1024], F32, tag="s2")
                        nc.tensor.matmul(s2[:, f0:512], kt_s[0:64, hp, ksl],
                                         qtc[0:64, hp, f0:],
                                         start=True, stop=True)
                        nc.tensor.matmul(s2[:, 512 + f0:1024],
                                         kt_s[64:128, hp, ksl],
                                         qtc[64:128, hp, f0:],
                                         start=True, stop=True)
                        s2s[kt] = s2

                    def softmax_pv(kt, remaining):
                        force_drain(kt // 4, "v", kt % 4)
                        s2 = s2s.pop(kt)
                        pt = pt_pool.tile([P, 2, 512], BF16, tag="pt")
                        di = kt - 4 * qc
                        # diagonal blocks: only columns >= f0 are causally
                        # visible; exp and PV restrict to them (kt==0 is
                        # always full-width, initializing every PSUM column
                        # of the PV accumulators). The diagonal 128-col
                        # block gets the additive -inf bias pre-exp.
                        f0 = max(0, di) * P
                        s2v = s2[:].rearrange("p (a b) -> p a b", a=2)
                        if f0 > 0:
                            nc.scalar.activation(
                                pt[:, :, f0:], s2v[:, :, f0:],
                                mybir.ActivationFunctionType.Exp,
                                scale=scale)
                        else:
                            nc.scalar.activation(
                                pt[:].rearrange("p a b -> p (a b)"), s2[:],
                                mybir.ActivationFunctionType.Exp,
                                scale=scale)
                        if di >= 0:
                            nc.vector.tensor_mul(
                                pt[:, :, f0:f0 + P], pt[:, :, f0:f0 + P],
                                mask[:, None, :].to_broadcast((P, 2, P)))
                        nc.tensor.matmul(pva[:, f0:], va[:, kt, hp, 0],
                                         pt[:, 0, f0:],
                                         start=(kt == 0), stop=(kt == nkt - 1))
                        nc.tensor.matmul(pvb[:, f0:], va[:, kt, hp, 1],
                                         pt[:, 1, f0:],
                                         start=(kt == 0), stop=(kt == nkt - 1))
                        if kt >= 2 and normtail:
                            normtail.pop(0)()
                        emit_fillers(remaining)

                    qk(0)
                    for kt in range(1, nkt):
                        qk(kt)
                        if kt == 1:
                            # cross-attend boundary: PV(0) waits on exp(0)
                            # and qk(2) on the s2 slot it frees — nothing
                            # attention-side can run, so force one filler in
                            emit_fillers(nkt - 1 + rem_after, boost=1.8)
                        softmax_pv(kt - 1, (nkt - kt) + rem_after)
                    softmax_pv(nkt - 1, 1 + rem_after)

                    final = hp == NHP - 1 and qc in (1, 3)
                    pvs = nrm_pool.tile([P, 2, 512], F32, tag="pvs")
                    rden = nrm_pool.tile([P, 2, 512], BF16, tag="rden")
                    if final:
                        # this tail gates a collective launch: shortest
                        # possible chain — reciprocals read the PSUM
                        # denominator rows directly and the copies split
                        # across DVE/ACT
                        with nc.allow_low_precision("f32r softmax denoms"):
                            nc.vector.reciprocal(rden[64:65, 0], pva[64:65])
                            nc.vector.reciprocal(rden[64:65, 1], pvb[64:65])
                        nc.vector.tensor_copy(pvs[0:65, 0], pva[:])
                        nc.scalar.copy(pvs[0:65, 1], pvb[:])
                        rba = ppool.tile([64, 512], F32, tag="proj")
                        rbb = ppool.tile([64, 512], F32, tag="proj")
                        nc.tensor.matmul(rba[:], ones64[64:65, :],
                                         rden[64:65, 0],
                                         start=True, stop=True)
                        nc.tensor.matmul(rbb[:], ones64[64:65, :],
                                         rden[64:65, 1],
                                         start=True, stop=True)
                        nc.vector.tensor_mul(aoq[0:64, hp], pvs[0:64, 0],
                                             rba[:])
                        nc.vector.tensor_mul(aoq[64:128, hp], pvs[0:64, 1],
                                             rbb[:])
                        dst = a2a_dst(qc, hp)
                        jsl = slice((qc % 2) * 4, (qc % 2) * 4 + 4)
                        nc.sync.dma_start(
                            dst[0:64, jsl],
                            aoq[0:64, hp].rearrange("p (j t) -> p j t", j=4))
                        nc.sync.dma_start(
                            dst[64:128, jsl],
                            aoq[64:128, hp].rearrange("p (j t) -> p j t",
                                                      j=4))
                        return
                    # copy PV accumulators out of PSUM fast (frees banks);
                    # defer the recip->broadcast->scale tail into the next
                    # head-pair's kt loop so PE never stalls behind it
                    nc.vector.tensor_copy(pvs[0:65, 0], pva[:])
                    nc.vector.tensor_copy(pvs[0:65, 1], pvb[:])
                    with nc.allow_low_precision("f32r softmax denominators"):
                        nc.vector.reciprocal(rden[64:65, 0], pvs[64:65, 0])
                        nc.vector.reciprocal(rden[64:65, 1], pvs[64:65, 1])

                    def tail(hp=hp, qc=qc, pvs=pvs, rden=rden):
                        # partition-broadcast the reciprocals via a DRAM
                        # bounce (stride-0 source): slower than a
                        # ones-matmul but entirely off the PE, and deferred
                        # tails have an attend's worth of slack
                        sl = (qc * NHP + hp) % 8
                        rb = nrm_pool.tile([64, 2, 512], BF16, tag="rb")
                        nc.sync.dma_start(rden_d[sl], rden[64:65, :, :])
                        nc.sync.dma_start(
                            rb[:], rden_d[sl:sl + 1].to_broadcast(
                                (64, 2, 512)))
                        nc.vector.tensor_mul(aoq[0:64, hp], pvs[0:64, 0],
                                             rb[:, 0])
                        nc.vector.tensor_mul(aoq[64:128, hp], pvs[0:64, 1],
                                             rb[:, 1])
                        # ship this head-pair's slice to the exchange buffer
                        # immediately so the collective's inputs aren't gated
                        # on one bulk DMA burst at stage end
                        nc.sync.dma_start(
                            a2a_dst(qc, hp)[:, (qc % 2) * 4:(qc % 2) * 4 + 4],
                            aoq[:, hp].rearrange("p (j t) -> p j t", j=4))
                    normtail.append(tail)

                # interleaved: project chunk tc, then attention q-chunk tc,
                # streaming each finished chunk into the re-shard buffers.
                # stage-2 row owner of q = m*1024 + j*128 + p is core j, so
                # the first collective can fire once q < 1024 is done.
                a2a_r0 = a2a_in0.rearrange("j (hp p) t -> p hp j t", p=P)
                a2a_r1a = a2a_in1a.rearrange("j (hp p) t -> p hp j t", p=P)
                a2a_r1b = a2a_in1b.rearrange("j (hp p) t -> p hp j t", p=P)

                def a2a_dst(qc, hp):
                    if qc // 2 == 0:
                        return a2a_r0[:, hp]
                    r = a2a_r1a if hp < 2 else a2a_r1b
                    return r[:, hp % 2]

                def emit_collective(cin, cout):
                    if sim:
                        nc.sync.dma_start(cout, cin)
                    else:
                        nc.gpsimd.collective_compute(
                            "AllToAll", mybir.AluOpType.bypass,
                            replica_groups=[list(range(NCORES))],
                            ins=[cin], outs=[cout])

                def load_aob(b, cout, nct):
                    aob = aob_pool.tile([P, nct, P], BF16, tag="aob")
                    nc.sync.dma_start(
                        aob[:],
                        cout[2 * b:2 * b + 2].rearrange(
                            "s (c p) t -> p (s c) t", p=P))
                    return aob

                def o_group(b, m, aob, osb, w, oc):
                    ps = ppool.tile([P, 512], F32, tag="proj")
                    for ct in range(NIT):
                        nc.tensor.matmul(
                            ps[:], aob[:, ct], w[:, ct],
                            start=(ct == 0), stop=(ct == NIT - 1))
                    osl = slice(oc * 512, (oc + 1) * 512)
                    nc.vector.tensor_copy(osb[:, osl], ps[:])
                    if oc == 1:
                        nc.sync.dma_start(out_d[b, m], osb[:])

                def o_stage(b, m, aob_get):
                    """Four labeled half-group filler entries (two per
                    512-col half of W_O) sharing one staging tile."""
                    slot = {}

                    def g(oc, half):
                        def f():
                            if "osb" not in slot:
                                slot["osb"] = osb_pool.tile(
                                    [P, D], F32, tag="osb",
                                    name=f"osb{m}_{b}")
                            w = (wo0, wo1)[oc]
                            aob = aob_get(b)
                            if half == 0:
                                slot[oc] = ppool.tile(
                                    [P, 512], F32, tag="proj",
                                    name=f"og{m}_{b}_{oc}")
                                for ct in range(NIT // 2):
                                    nc.tensor.matmul(
                                        slot[oc][:], aob[:, ct], w[:, ct],
                                        start=(ct == 0), stop=False)
                                return
                            ps = slot.pop(oc)
                            for ct in range(NIT // 2, NIT):
                                nc.tensor.matmul(
                                    ps[:], aob[:, ct], w[:, ct],
                                    start=False, stop=(ct == NIT - 1))
                            osl = slice(oc * 512, (oc + 1) * 512)
                            nc.vector.tensor_copy(slot["osb"][:, osl], ps[:])
                            if oc == 1:
                                nc.sync.dma_start(out_d[b, m],
                                                  slot["osb"][:])
                        return f
                    return [((9, "o", 2 * b + oc), g(oc, half), 853)
                            for oc in range(2) for half in range(2)]

                # m=1 output projection in two channel-halves: the hp0/1
                # contraction (W_O rows {0,1,4,5}) runs off collective 1a
                # as late-qc3 filler; the hp2/3 half accumulates on top
                # after collective 1b
                osb1 = {}
                HALF_A = (0, 1, 4, 5)
                HALF_B = (2, 3, 6, 7)

                def o_half(b, aob, oc, cts, first):
                    ps = ppool.tile([P, 512], F32, tag="proj")
                    for i, ct in enumerate(cts):
                        nc.tensor.matmul(
                            ps[:], aob[:, i], (wo0, wo1)[oc][:, ct],
                            start=(i == 0), stop=(i == len(cts) - 1))
                    osl = slice(oc * 512, (oc + 1) * 512)
                    if first:
                        nc.vector.tensor_copy(osb1[b][:, osl], ps[:])
                    else:
                        nc.vector.tensor_add(osb1[b][:, osl],
                                             osb1[b][:, osl], ps[:])
                        nc.sync.dma_start(out_d[b, 1, :, osl],
                                          osb1[b][:, osl])

                def o1a_stage(b, aob_get):
                    def g(oc):
                        def f():
                            if b not in osb1:
                                osb1[b] = osb_pool.tile(
                                    [P, D], F32, tag="osb",
                                    name=f"osb1_{b}")
                            o_half(b, aob_get(b), oc, HALF_A, True)
                        return f
                    return [((9, "oa", 2 * b + oc), g(oc), 852)
                            for oc in range(2)]

                aob0 = {}
                aob1a = {}
                aob1b = {}
                qtc = project(0, xtc=xtc0)
                # chunk 0 must fully project before attention starts
                while pending:
                    pop_front()
                for tc4 in range(NQC):
                    if tc4 + 1 < NQC:
                        next_qtc = project(tc4 + 1)  # queued as fillers
                    aoq = ao_pool.tile([P, NHP, 512], BF16, tag="aoq")
                    for hp in range(NHP):
                        if tc4 == 3 and hp == 2:
                            # hp0/1 tails of both m=1 chunks are shipped:
                            # fire the first m=1 half-collective and queue
                            # its output projection as late-qc3 filler
                            while normtail:
                                normtail.pop(0)()
                            emit_collective(a2a_in1a, a2a_out1a)
                            for b in range(B):
                                aob1a[b] = load_aob(b, a2a_out1a, 4)
                            for b in range(B):
                                for lb, fn, cost in o1a_stage(b, aob1a.get):
                                    push(lb, fn, cost)
                            reserve[0] = 11000
                        rem_after = (NHP - 1 - hp) * 4 * (tc4 + 1) + sum(
                            UNITS[tc4 + 1:])
                        attend(hp, tc4, qtc, aoq, rem_after)
                    if tc4 in (1, 3):
                        # the collective launch is gated on every tail DMA
                        # of its half; drain them now (fillers keep pacing)
                        while normtail:
                            normtail.pop(0)()
                    if tc4 == 1:
                        emit_collective(a2a_in0, a2a_out0)
                    if tc4 == 2:
                        # qc0/qc1 results finished resharding during qc2:
                        # W_O + the m=0 reshard tiles stream in now. The m=0
                        # output projection joins the filler queue behind
                        # chunk-3's projections, but at least 4 groups are
                        # held back (reserve) so PE has work after
                        # collective 1 is emitted, hiding the
                        # collective+reshard latency that gates m=1.
                        wo_r = wo_d.rearrange("(i p) o -> p i o", p=P)
                        nc.sync.dma_start(wo0[:], wo_r[:, :, 0:512])
                        nc.sync.dma_start(wo1[:], wo_r[:, :, 512:1024])
                        for b in range(B):
                            aob0[b] = load_aob(b, a2a_out0, NIT)
                        for b in range(B):
                            for lb, fn, cost in o_stage(b, 0, aob0.get):
                                push(lb, fn, cost)
                        reserve[0] = 11000
                    if tc4 + 1 < NQC:
                        qtc = next_qtc
                while pending and pcost[0] > reserve[0]:
                    pop_front()
                emit_collective(a2a_in1b, a2a_out1b)

                # m=1b reshard loads issue first: they only wait on the
                # collective, and queueing them behind other stores would
                # delay them on the in-order DMA queue
                for b in range(B):
                    aob1b[b] = load_aob(b, a2a_out1b, 4)

                # reserved groups hide the collective+reshard latency
                reserve[0] = 0.0
                while pending:
                    pop_front()

                # ---- m=1 hp2/3 half: accumulate + store ----------------
                for b in range(B):
                    for oc in range(2):
                        o_half(b, aob1b[b], oc, HALF_B, False)

    _split_multiwaits(nc)
    return nc


_NC_CACHE = None


def _get_nc():
    global _NC_CACHE
    if _NC_CACHE is None:
        _NC_CACHE = _build_nc()
    return _NC_CACHE


def make_in_maps(x, W_Q, W_K, W_V, W_O):
    bf = ml_dtypes.bfloat16
    wqt = np.ascontiguousarray(W_Q.T).astype(bf)
    wkt = np.ascontiguousarray(W_K.T).astype(bf)
    wvt = np.ascontiguousarray(W_V.T).astype(bf)
    wot = np.ascontiguousarray(W_O.T).astype(bf)
    in_maps = []
    for c in range(NCORES):
        b, g = c // 2, c % 2
        in_maps.append({
            "xt": np.ascontiguousarray(x[b].T).astype(bf),
            "wq": np.ascontiguousarray(wqt[:, g * CH:(g + 1) * CH]),
            "wk": np.ascontiguousarray(wkt[:, g * CH:(g + 1) * CH]),
            "wv": np.ascontiguousarray(wvt[:, g * CH:(g + 1) * CH]),
            "wo": wot,
        })
    return in_maps


def assemble(results):
    out = np.empty((B, T, D), np.float32)
    for j in range(NCORES):
        o = results[j]["out"]  # [B, 2, 128, D]
        for b in range(B):
            for m in range(2):
                r0 = m * 1024 + j * P
                out[b, r0:r0 + P, :] = o[b, m]
    return out


def kernel(x, W_Q, W_K, W_V, W_O):
    x = np.asarray(x, np.float32)
    in_maps = make_in_maps(x, np.asarray(W_Q, np.float32),
                           np.asarray(W_K, np.float32),
                           np.asarray(W_V, np.float32),
                           np.asarray(W_O, np.float32))
    nc = _get_nc()
    res = run_bass_kernel_spmd(nc, in_maps, core_ids=list(range(NCORES)))
    return assemble(res.results)


# revision 99
# speedup vs baseline: 1.2452x; 1.0304x over previous
"""Causal multi-head attention (B=4, T=2048, D=1024, H=16) on 8 NeuronCores.

Sharding:
  stage 1 (QKV proj + attention): core c -> batch c//2, head-group c%2
    (8 of 16 heads, 512 of 1024 channels). Data-parallel on B, tensor-
    parallel on heads.
  stage 2 (output projection): one 8-rank AllToAll re-shards attention
    output to (all 4 batches x 256-token t-slice) per core, then each core
    computes out = attn_out @ W_O.T for its 1024 rows. No reduction needed.

Matmul operands are bf16 (same PE rate as fp32r at >=256 moving columns,
full rate below it, half the DMA/SBUF footprint); PSUM accumulation stays
fp32. exp runs on the scalar engine reading PSUM directly with the softmax
scale fused; the softmax denominator comes for free as a 65th output row of
the PV matmul (V augmented with a ones column). Causal masking multiplies
the 128-col diagonal block by a single block-local 0/1 mask. Softmax
reciprocals are partition-broadcast through a DRAM bounce buffer on
deferred tails (zero PE cost) and by ones-matmuls on the two
collective-gating tails (lowest latency).

Scheduling: attention is ACT-bound per k-tile (exp ~1040ns vs QK+PV
~850ns), so projection work for the NEXT chunk is queued and paced into the
attention stream as PE filler. The filler queue is labeled with
(chunk, kind, index) and force-drained at each consumer site (Q before the
attend that reads it, K/V before the k-tile that reads them), which lets
the pacer spread filler across the whole remaining schedule without
correctness risk; a fixed-boost pop at each attend boundary covers the
software-pipeline refill bubble. Large input loads are batched into few
descriptor-heavy DMAs (HWDGE charges a fixed ~625ns per DMA instruction).
The endgame keeps PE fed through both collectives: the m=0 output
projection and the hp0/1 half of the m=1 projection (split AllToAll) run
as late-qc3 + post-collective filler, with ~12us of queue held back to
cover the final collective+reshard latency that gates the m=1 hp2/3 half.
"""
import numpy as np
import ml_dtypes

import concourse.bass as bass
import concourse.mybir as mybir
import concourse.tile as tile
from concourse.bass_utils import run_bass_kernel_spmd

F32 = mybir.dt.float32
F32R = mybir.dt.float32r
BF16 = mybir.dt.bfloat16

P = 128
B, T, D = 4, 2048, 1024
H, HD = 16, 64
NCORES = 8
CH = D // 2          # channels per core (8 heads)
NHP = 4              # head pairs per core
NKT = T // P         # 16 k-tiles
NQC = T // 512       # 4 q-chunks
NIT = D // P         # 8 input-dim tiles


def _split_multiwaits(nc) -> int:
    """walrus here rejects >1 sem wait per instruction; split extras into
    wait-only NoOps on the same engine."""
    nsplit = 0
    for f in nc.m.functions:
        for bb in f.blocks:
            if not any(
                i.sync_info is not None and i.sync_info.on_wait is not None
                and len(i.sync_info.on_wait) > 1 for i in bb.instructions
            ):
                continue
            new_list = []
            for inst in bb.instructions:
                si = inst.sync_info
                if si is not None and si.on_wait is not None and len(si.on_wait) > 1:
                    waits = list(si.on_wait)
                    for k, w in enumerate(waits[:-1]):
                        n = mybir.InstNoOp(
                            name=f"{inst.name}-wsplit{k}", ins=[], outs=[])
                        n.engine = inst.engine
                        n.sync_info = mybir.SyncInfo(on_wait=[w], on_update=[])
                        new_list.append(n)
                        nsplit += 1
                    inst.sync_info = mybir.SyncInfo(
                        on_wait=[waits[-1]], on_update=list(si.on_update or []))
                new_list.append(inst)
            bb.instructions = new_list
    return nsplit


def _build_nc(sim: bool = False):
    nc = bass.Bass("TRN2", target_bir_lowering=False, debug=False,
                   num_devices=NCORES)
    xt_d = nc.dram_tensor("xt", [D, T], BF16, kind="ExternalInput").ap()
    wq_d = nc.dram_tensor("wq", [D, CH], BF16, kind="ExternalInput").ap()
    wk_d = nc.dram_tensor("wk", [D, CH], BF16, kind="ExternalInput").ap()
    wv_d = nc.dram_tensor("wv", [D, CH], BF16, kind="ExternalInput").ap()
    wo_d = nc.dram_tensor("wo", [D, D], BF16, kind="ExternalInput").ap()
    out_d = nc.dram_tensor("out", [B, 2, P, D], BF16,
                       kind="ExternalOutput").ap()
    a2a_in0 = nc.dram_tensor("a2a_in0", [NCORES, CH, P], BF16).ap()
    a2a_out0 = nc.dram_tensor("a2a_out0", [NCORES, CH, P], BF16).ap()
    # the m=1 exchange is split by head-pair half: the hp0/1 half fires
    # mid-qc3 (its tails are done) so half the m=1 output projection is
    # available as late-qc3 filler; only the hp2/3 half gates the end
    a2a_in1a = nc.dram_tensor("a2a_in1a", [NCORES, CH // 2, P], BF16).ap()
    a2a_out1a = nc.dram_tensor("a2a_out1a", [NCORES, CH // 2, P], BF16).ap()
    a2a_in1b = nc.dram_tensor("a2a_in1b", [NCORES, CH // 2, P], BF16).ap()
    a2a_out1b = nc.dram_tensor("a2a_out1b", [NCORES, CH // 2, P], BF16).ap()
    # bounce buffer for the softmax-reciprocal partition broadcast: DMA a
    # [1,2,512] row out and re-read it with a stride-0 partition source
    # (direct SBUF->SBUF partition broadcast is not expressible; engines
    # would need a ones-matmul, which costs PE rows on deferred tails)
    rden_d = nc.dram_tensor("rden_d", [8, 2, 512], BF16).ap()

    scale = float(1.0 / np.sqrt(HD))
    # attention units (k-tiles) per q-chunk, for filler pacing
    UNITS = [4 * (qc + 1) * NHP for qc in range(NQC)]

    with tile.TileContext(nc) as tc:
        with (
            tc.tile_pool(name="persist", bufs=1) as persist,
        ):
            # ---- persistent SBUF tensors -------------------------------
            kt_s = persist.tile([P, NHP, T], BF16)    # K^T  (channels, k)
            va = persist.tile([P, NKT, NHP, 2, HD + 1], BF16)  # V | ones

            with (
                tc.tile_pool(name="wpool", bufs=1) as wpool,
                tc.tile_pool(name="xpool", bufs=2) as xpool,
                tc.tile_pool(name="aob_pool", bufs=8) as aob_pool,
                tc.tile_pool(name="osb_pool", bufs=8) as osb_pool,
                tc.tile_pool(name="qpool", bufs=2) as qpool,
                tc.tile_pool(name="ao_pool", bufs=2) as ao_pool,
                tc.tile_pool(name="mpool", bufs=1) as mpool,
                tc.tile_pool(name="pt_pool", bufs=16) as pt_pool,
                tc.tile_pool(name="nrm_pool", bufs=3) as nrm_pool,
                tc.tile_pool(name="ppool", bufs=2, space="PSUM") as ppool,
                tc.tile_pool(name="ps_s", bufs=2, space="PSUM") as ps_s,
                tc.tile_pool(name="ps_pv", bufs=1, space="PSUM") as ps_pv,
            ):
                wq = wpool.tile([P, NIT, CH], BF16)
                wk = wpool.tile([P, NIT, CH], BF16)
                wv = wpool.tile([P, NIT, CH], BF16)
                wo0 = wpool.tile([P, NIT, 512], BF16)
                wo1 = wpool.tile([P, NIT, 512], BF16)
                xt_r = xt_d.rearrange("(i p) t -> p i t", p=P)
                xtc0 = xpool.tile([P, NIT, 512], BF16, tag="xtc")
                # staged arrival: it=0 first (smallest useful unit), then
                # batched remainders — descriptor-heavy DMAs beat
                # instruction-overhead-bound ones (fixed HWDGE cost per
                # DMA), and x/wv interleave because the V matmuls for tile
                # `it` need both tensors' slices and the DMA device is
                # serial
                wv_r = wv_d.rearrange("(i p) o -> p i o", p=P)
                nc.sync.dma_start(xtc0[:, 0], xt_r[:, 0, 0:512])
                nc.sync.dma_start(wv[:, 0], wv_r[:, 0])
                nc.sync.dma_start(xtc0[:, 1:4], xt_r[:, 1:4, 0:512])
                nc.sync.dma_start(wv[:, 1:4], wv_r[:, 1:4])
                nc.sync.dma_start(xtc0[:, 4:], xt_r[:, 4:, 0:512])
                nc.sync.dma_start(wv[:, 4:], wv_r[:, 4:])
                nc.sync.dma_start(wq[:], wq_d.rearrange(
                    "(i p) o -> p i o", p=P))
                nc.sync.dma_start(wk[:], wk_d.rearrange(
                    "(i p) o -> p i o", p=P))

                # ones: stationary rows for the denominator broadcast
                # matmuls + broadcast-copy source for the V|ones column
                # (a strided bf16 memset into va fails the ISA memset
                # value-type check, so fill via ACT broadcast copy)
                ones64 = mpool.tile([P, 64], BF16, tag="ones64")
                nc.gpsimd.memset(ones64[:], 1.0)

                # p-state warmup: the tensor engine runs at half clock until
                # it has been continuously busy for 3us. Burn that ramp on
                # dummy matmuls during the initial DMA wait so the real
                # projections start at full speed.
                warm = mpool.tile([1, 512], BF16, tag="warm")
                nc.vector.memset(warm[:], 1.0)
                wps = ppool.tile([64, 512], F32, tag="proj")
                for _ in range(12):
                    nc.tensor.matmul(wps[:, 0:128], warm[0:1, 0:64],
                                     warm[0:1, 0:128],
                                     start=True, stop=True)
                nc.scalar.copy(
                    va[:, :, :, :, HD],
                    ones64[:, 0:1].to_broadcast((P, NKT, NHP, 2)))
                # 0/1 causal mask for the 128-col diagonal block: in
                # block-local coords the visible region is q_local >= p for
                # every diagonal tile, so one tile serves all of them
                mask = mpool.tile([P, P], BF16, tag="mask")
                nc.gpsimd.memset(mask[:], 1.0)
                nc.gpsimd.affine_select(
                    out=mask[:], in_=mask[:],
                    compare_op=mybir.AluOpType.is_ge,
                    fill=0.0, base=0, channel_multiplier=-1,
                    pattern=[[1, P]])

                # pending projection psum-groups of upcoming chunks, emitted
                # as PE filler work inside the attention kt loops. Entries
                # are (label, fn) with label=(chunk, kind, idx) so consumer
                # sites can force-drain exactly what they depend on.
                pending = []    # (label, fn, cost_ns)
                normtail = []
                filler_acc = [0.0]
                pcost = [0.0]
                reserve = [0.0]  # ns of work kept for the post-collective gap

                def push(label, fn, cost):
                    pending.append((label, fn, cost))
                    pcost[0] += cost

                def pop_front():
                    lb, fn, cost = pending.pop(0)
                    pcost[0] -= cost
                    fn()

                def emit_fillers(remaining_units, boost=0.0):
                    # proportional pacing: spread the queue across the whole
                    # remaining schedule instead of draining it greedily
                    # (late ACT-bound units would idle PE). `boost` forces
                    # extra pops at known PE-stall sites.
                    if not pending or pcost[0] <= reserve[0]:
                        return
                    filler_acc[0] += boost + len(pending) / max(
                        1, remaining_units)
                    while (filler_acc[0] >= 1.0 and pending
                           and pcost[0] > reserve[0]):
                        filler_acc[0] -= 1.0
                        pop_front()

                def force_drain(chunk, kind, idx):
                    """Pop fillers (in order) until no queued entry matches
                    (chunk, kind, <=idx) — consumer is about to read them."""
                    while any(lb[0] == chunk and lb[1] == kind and lb[2] <= idx
                              for lb, _, _ in pending):
                        pop_front()

                def project(tc4, xtc=None):
                    """Queue QKV projection psum-groups for t-chunk tc4.
                    Returns the Q^T chunk tile; the groups themselves are
                    emitted later as PE filler inside attention."""
                    if xtc is None:
                        xtc = xpool.tile([P, NIT, 512], BF16, tag="xtc")
                        nc.sync.dma_start(
                            xtc[:], xt_r[:, :, tc4 * 512:(tc4 + 1) * 512])
                    qtc = qpool.tile([P, NHP, 512], BF16, tag="qtc")

                    def qk_group(w, dst, dsl, ot):
                        # two half-contraction pieces sharing one psum tile:
                        # finer filler quanta track the per-k-tile PE deficit
                        # during ACT-bound attention much more closely
                        st = {}

                        def ga():
                            st["ps"] = ppool.tile([P, 512], F32, tag="proj",
                                                  name=f"qk{tc4}_{ot}")
                            for it in range(NIT // 2):
                                nc.tensor.matmul(
                                    st["ps"][:], w[:, it, ot * P:(ot + 1) * P],
                                    xtc[:, it], start=(it == 0), stop=False)

                        def gb():
                            ps = st["ps"]
                            for it in range(NIT // 2, NIT):
                                nc.tensor.matmul(
                                    ps[:], w[:, it, ot * P:(ot + 1) * P],
                                    xtc[:, it], start=False,
                                    stop=(it == NIT - 1))
                            nc.vector.tensor_copy(dst[:, ot, dsl], ps[:])
                        return ga, gb

                    def v_group(tt4):
                        st = {}

                        def ga():
                            st["ps"] = ppool.tile([P, 512], F32, tag="proj",
                                                  name=f"v{tc4}_{tt4}")
                            for it in range(NIT // 2):
                                nc.tensor.matmul(
                                    st["ps"][:],
                                    xtc[:, it, tt4 * P:(tt4 + 1) * P],
                                    wv[:, it], start=(it == 0), stop=False)

                        def gb():
                            ps = st["ps"]
                            for it in range(NIT // 2, NIT):
                                nc.tensor.matmul(
                                    ps[:], xtc[:, it, tt4 * P:(tt4 + 1) * P],
                                    wv[:, it], start=False,
                                    stop=(it == NIT - 1))
                            nc.vector.tensor_copy(
                                va[:, tc4 * 4 + tt4, :, :, 0:HD],
                                ps[:].rearrange("p (hp h d) -> p hp h d",
                                                hp=NHP, h=2))
                        return ga, gb

                    if tc4 == 0:
                        # V first: wv+x arrive first and the four V groups
                        # run it-major across four concurrent psums, so each
                        # arriving (x, wv) DMA chunk feeds 4 matmuls instead
                        # of 1 during the DMA-bound startup ramp
                        def v_block0():
                            pss = [
                                ppool.tile([P, 512], F32, tag="proj",
                                           name="v0ps0"),
                                ppool.tile([P, 512], F32, tag="proj",
                                           name="v0ps1"),
                                ps_s.tile([P, 512], F32, tag="s2",
                                          name="v0ps2"),
                                ps_s.tile([P, 512], F32, tag="s2",
                                          name="v0ps3"),
                            ]
                            for it in range(NIT):
                                for tt4 in range(4):
                                    nc.tensor.matmul(
                                        pss[tt4][:],
                                        xtc[:, it, tt4 * P:(tt4 + 1) * P],
                                        wv[:, it], start=(it == 0),
                                        stop=(it == NIT - 1))
                            for tt4 in range(4):
                                nc.vector.tensor_copy(
                                    va[:, tt4, :, :, 0:HD],
                                    pss[tt4][:].rearrange(
                                        "p (hp h d) -> p hp h d",
                                        hp=NHP, h=2))
                        push((0, "v", 3), v_block0, 6816)
                        for ot in range(NHP):
                            for g in qk_group(wq, qtc, slice(0, 512), ot):
                                push((0, "q", ot), g, 853)
                            for g in qk_group(
                                    wk, kt_s,
                                    slice(tc4 * 512, (tc4 + 1) * 512), ot):
                                push((0, "k", ot), g, 853)
                    else:
                        # interleave Q/K per head-pair so force-drain
                        # deadlines pop the minimum prefix
                        for ot in range(NHP):
                            for g in qk_group(wq, qtc, slice(0, 512), ot):
                                push((tc4, "q", ot), g, 853)
                            for g in qk_group(
                                    wk, kt_s,
                                    slice(tc4 * 512, (tc4 + 1) * 512), ot):
                                push((tc4, "k", ot), g, 853)
                        for tt4 in range(4):
                            for g in v_group(tt4):
                                push((tc4, "v", tt4), g, 853)
                    return qtc

                def attend(hp, qc, qtc, aoq, rem_after):
                    """Attention for head-pair hp, q-chunk qc. kt loop is
                    software-pipelined: QK(kt+1) issues before PV(kt) so PE
                    isn't stalled behind the exp of the current tile."""
                    nkt = 4 * (qc + 1)
                    final = hp == NHP - 1 and qc in (1, 3)
                    force_drain(qc, "q", hp)
                    pva = ps_pv.tile([HD + 1, 512], F32, tag="pva")
                    pvb = ps_pv.tile([HD + 1, 512], F32, tag="pvb")
                    s2s = {}

                    def qk(kt):
                        force_drain(kt // 4, "k", hp)
                        ksl = slice(kt * P, (kt + 1) * P)
                        f0 = max(0, kt - 4 * qc) * P  # first visible q column
                        s2 = ps_s.tile([P, 1024], F32, tag="s2")
                        nc.tensor.matmul(s2[:, f0:512], kt_s[0:64, hp, ksl],
                                         qtc[0:64, hp, f0:],
                                         start=True, stop=True)
                        nc.tensor.matmul(s2[:, 512 + f0:1024],
                                         kt_s[64:128, hp, ksl],
                                         qtc[64:128, hp, f0:],
                                         start=True, stop=True)
                        s2s[kt] = s2

                    def softmax_pv(kt, remaining):
                        force_drain(kt // 4, "v", kt % 4)
                        s2 = s2s.pop(kt)
                        pt = pt_pool.tile([P, 2, 512], BF16, tag="pt")
                        di = kt - 4 * qc
                        # diagonal blocks: only columns >= f0 are causally
                        # visible; exp and PV restrict to them (kt==0 is
                        # always full-width, initializing every PSUM column
                        # of the PV accumulators). The diagonal 128-col
                        # block gets the additive -inf bias pre-exp.
                        f0 = max(0, di) * P
                        s2v = s2[:].rearrange("p (a b) -> p a b", a=2)
                        if f0 > 0:
                            nc.scalar.activation(
                                pt[:, :, f0:], s2v[:, :, f0:],
                                mybir.ActivationFunctionType.Exp,
                                scale=scale)
                        else:
                            nc.scalar.activation(
                                pt[:].rearrange("p a b -> p (a b)"), s2[:],
                                mybir.ActivationFunctionType.Exp,
                                scale=scale)
                        if di >= 0:
                            nc.vector.tensor_mul(
                                pt[:, :, f0:f0 + P], pt[:, :, f0:f0 + P],
                                mask[:, None, :].to_broadcast((P, 2, P)))
                        # filler ahead of the PV pair in program order: if
                        # PV head-of-line-blocks on this tile's exp, the
                        # filler already in the queue runs during the wait
                        emit_fillers(remaining)
                        nc.tensor.matmul(pva[:, f0:], va[:, kt, hp, 0],
                                         pt[:, 0, f0:],
                                         start=(kt == 0), stop=(kt == nkt - 1))
                        nc.tensor.matmul(pvb[:, f0:], va[:, kt, hp, 1],
                                         pt[:, 1, f0:],
                                         start=(kt == 0), stop=(kt == nkt - 1))
                        if kt >= 2 and normtail:
                            normtail.pop(0)()
                        if final and kt >= nkt - 2 and pending:
                            # the reserve normally blocks pops here, but on
                            # a collective-gating attend the diagonal
                            # cascade idles PE and the held work runs
                            # before the collective either way — spend it
                            # at the stall sites
                            pop_front()

                    qk(0)
                    for kt in range(1, nkt):
                        qk(kt)
                        if kt == 1:
                            # cross-attend boundary: PV(0) waits on exp(0)
                            # and qk(2) on the s2 slot it frees — nothing
                            # attention-side can run, so force one filler in
                            emit_fillers(nkt - 1 + rem_after, boost=1.4)
                        softmax_pv(kt - 1, (nkt - kt) + rem_after)
                    softmax_pv(nkt - 1, 1 + rem_after)

                    pvs = nrm_pool.tile([P, 2, 512], BF16, tag="pvs")
                    rden = nrm_pool.tile([P, 2, 512], BF16, tag="rden")
                    if final:
                        # this tail gates a collective launch: shortest
                        # possible chain — reciprocals read the PSUM
                        # denominator rows directly and the copies split
                        # across DVE/ACT
                        with nc.allow_low_precision("f32r softmax denoms"):
                            nc.vector.reciprocal(rden[64:65, 0], pva[64:65])
                            nc.vector.reciprocal(rden[64:65, 1], pvb[64:65])
                        # the rb matmuls below head-of-line-block the
                        # in-order PE queue while waiting on the recips;
                        # slot filler matmuls in front so PE chews through
                        # the reciprocal chain instead of idling
                        for _ in range(4):
                            if pending:
                                pop_front()
                        # both copies on ACT (idle at stage end): DVE's
                        # serial chain here is recips -> muls, and every op
                        # moved off it launches the collective sooner
                        nc.scalar.copy(pvs[0:65, 0], pva[:])
                        nc.scalar.copy(pvs[0:65, 1], pvb[:])
                        rba = ppool.tile([64, 512], F32, tag="proj")
                        rbb = ppool.tile([64, 512], F32, tag="proj")
                        nc.tensor.matmul(rba[:], ones64[64:65, :],
                                         rden[64:65, 0],
                                         start=True, stop=True)
                        nc.tensor.matmul(rbb[:], ones64[64:65, :],
                                         rden[64:65, 1],
                                         start=True, stop=True)
                        nc.vector.tensor_mul(aoq[0:64, hp], pvs[0:64, 0],
                                             rba[:])
                        nc.vector.tensor_mul(aoq[64:128, hp], pvs[0:64, 1],
                                             rbb[:])
                        dst = a2a_dst(qc, hp)
                        jsl = slice((qc % 2) * 4, (qc % 2) * 4 + 4)
                        nc.sync.dma_start(
                            dst[0:64, jsl],
                            aoq[0:64, hp].rearrange("p (j t) -> p j t", j=4))
                        nc.sync.dma_start(
                            dst[64:128, jsl],
                            aoq[64:128, hp].rearrange("p (j t) -> p j t",
                                                      j=4))
                        return
                    # copy PV accumulators out of PSUM fast (frees banks);
                    # defer the recip->broadcast->scale tail into the next
                    # head-pair's kt loop so PE never stalls behind it
                    nc.vector.tensor_copy(pvs[0:65, 0], pva[:])
                    nc.vector.tensor_copy(pvs[0:65, 1], pvb[:])
                    with nc.allow_low_precision("f32r softmax denominators"):
                        nc.vector.reciprocal(rden[64:65, 0], pvs[64:65, 0])
                        nc.vector.reciprocal(rden[64:65, 1], pvs[64:65, 1])

                    def tail(hp=hp, qc=qc, pvs=pvs, rden=rden):
                        # partition-broadcast the reciprocals via a DRAM
                        # bounce (stride-0 source): slower than a
                        # ones-matmul but entirely off the PE, and deferred
                        # tails have an attend's worth of slack
                        sl = (qc * NHP + hp) % 8
                        rb = nrm_pool.tile([64, 2, 512], BF16, tag="rb")
                        nc.sync.dma_start(rden_d[sl], rden[64:65, :, :])
                        nc.sync.dma_start(
                            rb[:], rden_d[sl:sl + 1].to_broadcast(
                                (64, 2, 512)))
                        nc.vector.tensor_mul(aoq[0:64, hp], pvs[0:64, 0],
                                             rb[:, 0])
                        nc.vector.tensor_mul(aoq[64:128, hp], pvs[0:64, 1],
                                             rb[:, 1])
                        # ship this head-pair's slice to the exchange buffer
                        # immediately so the collective's inputs aren't gated
                        # on one bulk DMA burst at stage end
                        nc.sync.dma_start(
                            a2a_dst(qc, hp)[:, (qc % 2) * 4:(qc % 2) * 4 + 4],
                            aoq[:, hp].rearrange("p (j t) -> p j t", j=4))
                    normtail.append(tail)

                # interleaved: project chunk tc, then attention q-chunk tc,
                # streaming each finished chunk into the re-shard buffers.
                # stage-2 row owner of q = m*1024 + j*128 + p is core j, so
                # the first collective can fire once q < 1024 is done.
                a2a_r0 = a2a_in0.rearrange("j (hp p) t -> p hp j t", p=P)
                a2a_r1a = a2a_in1a.rearrange("j (hp p) t -> p hp j t", p=P)
                a2a_r1b = a2a_in1b.rearrange("j (hp p) t -> p hp j t", p=P)

                def a2a_dst(qc, hp):
                    if qc // 2 == 0:
                        return a2a_r0[:, hp]
                    r = a2a_r1a if hp < 2 else a2a_r1b
                    return r[:, hp % 2]

                def emit_collective(cin, cout):
                    if sim:
                        nc.sync.dma_start(cout, cin)
                    else:
                        nc.gpsimd.collective_compute(
                            "AllToAll", mybir.AluOpType.bypass,
                            replica_groups=[list(range(NCORES))],
                            ins=[cin], outs=[cout])

                def load_aob(b, cout, nct):
                    aob = aob_pool.tile([P, nct, P], BF16, tag="aob")
                    nc.sync.dma_start(
                        aob[:],
                        cout[2 * b:2 * b + 2].rearrange(
                            "s (c p) t -> p (s c) t", p=P))
                    return aob

                def o_group(b, m, aob, osb, w, oc):
                    ps = ppool.tile([P, 512], F32, tag="proj")
                    for ct in range(NIT):
                        nc.tensor.matmul(
                            ps[:], aob[:, ct], w[:, ct],
                            start=(ct == 0), stop=(ct == NIT - 1))
                    osl = slice(oc * 512, (oc + 1) * 512)
                    nc.vector.tensor_copy(osb[:, osl], ps[:])
                    if oc == 1:
                        nc.sync.dma_start(out_d[b, m], osb[:])

                def o_stage(b, m, aob_get):
                    """Four labeled half-group filler entries (two per
                    512-col half of W_O) sharing one staging tile."""
                    slot = {}

                    def g(oc, half):
                        def f():
                            if "osb" not in slot:
                                slot["osb"] = osb_pool.tile(
                                    [P, D], BF16, tag="osb",
                                    name=f"osb{m}_{b}")
                            w = (wo0, wo1)[oc]
                            aob = aob_get(b)
                            if half == 0:
                                slot[oc] = ppool.tile(
                                    [P, 512], F32, tag="proj",
                                    name=f"og{m}_{b}_{oc}")
                                for ct in range(NIT // 2):
                                    nc.tensor.matmul(
                                        slot[oc][:], aob[:, ct], w[:, ct],
                                        start=(ct == 0), stop=False)
                                return
                            ps = slot.pop(oc)
                            for ct in range(NIT // 2, NIT):
                                nc.tensor.matmul(
                                    ps[:], aob[:, ct], w[:, ct],
                                    start=False, stop=(ct == NIT - 1))
                            osl = slice(oc * 512, (oc + 1) * 512)
                            nc.vector.tensor_copy(slot["osb"][:, osl], ps[:])
                            if oc == 1:
                                nc.sync.dma_start(out_d[b, m],
                                                  slot["osb"][:])
                        return f
                    return [((9, "o", 2 * b + oc), g(oc, half), 853)
                            for oc in range(2) for half in range(2)]

                # m=1 output projection in two channel-halves: the hp0/1
                # contraction (W_O rows {0,1,4,5}) runs off collective 1a
                # as late-qc3 filler; the hp2/3 half accumulates on top
                # after collective 1b
                osb1 = {}
                HALF_A = (0, 1, 4, 5)
                HALF_B = (2, 3, 6, 7)

                def o_half(b, aob, oc, cts, first):
                    ps = ppool.tile([P, 512], F32, tag="proj")
                    for i, ct in enumerate(cts):
                        nc.tensor.matmul(
                            ps[:], aob[:, i], (wo0, wo1)[oc][:, ct],
                            start=(i == 0), stop=(i == len(cts) - 1))
                    osl = slice(oc * 512, (oc + 1) * 512)
                    if first:
                        nc.vector.tensor_copy(osb1[b][:, osl], ps[:])
                    else:
                        nc.vector.tensor_add(osb1[b][:, osl],
                                             osb1[b][:, osl], ps[:])
                        nc.sync.dma_start(out_d[b, 1, :, osl],
                                          osb1[b][:, osl])

                def o1a_stage(b, aob_get):
                    def g(oc):
                        def f():
                            if b not in osb1:
                                osb1[b] = osb_pool.tile(
                                    [P, D], BF16, tag="osb",
                                    name=f"osb1_{b}")
                            o_half(b, aob_get(b), oc, HALF_A, True)
                        return f
                    return [((9, "oa", 2 * b + oc), g(oc), 852)
                            for oc in range(2)]

                aob0 = {}
                aob1a = {}
                aob1b = {}
                qtc = project(0, xtc=xtc0)
                # chunk 0 must fully project before attention starts
                while pending:
                    pop_front()
                for tc4 in range(NQC):
                    if tc4 + 1 < NQC:
                        next_qtc = project(tc4 + 1)  # queued as fillers
                    aoq = ao_pool.tile([P, NHP, 512], BF16, tag="aoq")
                    for hp in range(NHP):
                        if tc4 == 3 and hp == 2:
                            # hp0/1 tails of both m=1 chunks are shipped:
                            # fire the first m=1 half-collective and queue
                            # its output projection as late-qc3 filler
                            while normtail:
                                normtail.pop(0)()
                            emit_collective(a2a_in1a, a2a_out1a)
                            for b in range(B):
                                aob1a[b] = load_aob(b, a2a_out1a, 4)
                            for b in range(B):
                                for lb, fn, cost in o1a_stage(b, aob1a.get):
                                    push(lb, fn, cost)
                            reserve[0] = 17500
                        rem_after = (NHP - 1 - hp) * 4 * (tc4 + 1) + sum(
                            UNITS[tc4 + 1:])
                        attend(hp, tc4, qtc, aoq, rem_after)
                    if tc4 in (1, 3):
                        # the collective launch is gated on every tail DMA
                        # of its half; drain them now (fillers keep pacing)
                        while normtail:
                            normtail.pop(0)()
                    if tc4 == 1:
                        emit_collective(a2a_in0, a2a_out0)
                    if tc4 == 2:
                        # qc0/qc1 results finished resharding during qc2:
                        # W_O + the m=0 reshard tiles stream in now. The m=0
                        # output projection joins the filler queue behind
                        # chunk-3's projections, but at least 4 groups are
                        # held back (reserve) so PE has work after
                        # collective 1 is emitted, hiding the
                        # collective+reshard latency that gates m=1.
                        wo_r = wo_d.rearrange("(i p) o -> p i o", p=P)
                        nc.sync.dma_start(wo0[:], wo_r[:, :, 0:512])
                        nc.sync.dma_start(wo1[:], wo_r[:, :, 512:1024])
                        for b in range(B):
                            aob0[b] = load_aob(b, a2a_out0, NIT)
                        for b in range(B):
                            for lb, fn, cost in o_stage(b, 0, aob0.get):
                                push(lb, fn, cost)
                        reserve[0] = 17500
                    if tc4 + 1 < NQC:
                        qtc = next_qtc
                # release most of the reserve: from here the remaining
                # queue runs after the collective emission in program
                # order, so holding more than the collective+reshard
                # latency only delays the m=1 hp2/3 half
                reserve[0] = 17500
                while pending and pcost[0] > reserve[0]:
                    pop_front()
                emit_collective(a2a_in1b, a2a_out1b)

                # m=1b reshard loads issue first: they only wait on the
                # collective, and queueing them behind other stores would
                # delay them on the in-order DMA queue
                for b in range(B):
                    aob1b[b] = load_aob(b, a2a_out1b, 4)

                # reserved groups hide the collective+reshard latency
                reserve[0] = 0.0
                while pending:
                    pop_front()

                # ---- m=1 hp2/3 half: accumulate + store ----------------
                for b in range(B):
                    for oc in range(2):
                        o_half(b, aob1b[b], oc, HALF_B, False)

    _split_multiwaits(nc)
    return nc


_NC_CACHE = None


def _get_nc():
    global _NC_CACHE
    if _NC_CACHE is None:
        _NC_CACHE = _build_nc()
    return _NC_CACHE


def make_in_maps(x, W_Q, W_K, W_V, W_O):
    bf = ml_dtypes.bfloat16
    wqt = np.ascontiguousarray(W_Q.T).astype(bf)
    wkt = np.ascontiguousarray(W_K.T).astype(bf)
    wvt = np.ascontiguousarray(W_V.T).astype(bf)
    wot = np.ascontiguousarray(W_O.T).astype(bf)
    in_maps = []
    for c in range(NCORES):
        b, g = c // 2, c % 2
        in_maps.append({
            "xt": np.ascontiguousarray(x[b].T).astype(bf),
            "wq": np.ascontiguousarray(wqt[:, g * CH:(g + 1) * CH]),
            "wk": np.ascontiguousarray(wkt[:, g * CH:(g + 1) * CH]),
            "wv": np.ascontiguousarray(wvt[:, g * CH:(g + 1) * CH]),
            "wo": wot,
        })
    return in_maps


def assemble(results):
    out = np.empty((B, T, D), np.float32)
    for j in range(NCORES):
        o = np.asarray(results[j]["out"], np.float32)  # [B, 2, 128, D]
        for b in range(B):
            for m in range(2):
                r0 = m * 1024 + j * P
                out[b, r0:r0 + P, :] = o[b, m]
    return out


def kernel(x, W_Q, W_K, W_V, W_O):
    x = np.asarray(x, np.float32)
    in_maps = make_in_maps(x, np.asarray(W_Q, np.float32),
                           np.asarray(W_K, np.float32),
                           np.asarray(W_V, np.float32),
                           np.asarray(W_O, np.float32))
    nc = _get_nc()
    res = run_bass_kernel_spmd(nc, in_maps, core_ids=list(range(NCORES)))
    return assemble(res.results)
